# revision 8
# baseline (speedup 1.0000x reference)
"""CapsNet dynamic-routing layer on 8 Trainium2 NeuronCores (Bass/Tile), v2.

reference math (per batch element b):
  u_hat[b,i,o,j] = sum_d W[i,o,j,d] * u[b,i,d]        (never materialized)
  bl = 0; for r in 0..2:
    c = softmax_o(bl); s[b,o,j] = sum_i c*u_hat; v = squash(s)
    if r < 2: bl += sum_j u_hat*v
  return v  [B, 10, 16]

Distribution: pure data parallel, batch 512 -> 64 per core x 8 cores;
weights replicated.  Per-core: b=64, i=1152=9*128, o=10, j=16, d=8.

v2 design vs v1:
  * m1 (s-matmuls) output b-partitioned [64, (o,j)] -> n=16/o per
    instruction instead of n=64: 4x fewer PE rows.
  * m2 (agreement) G^T = W^T v computed (d,i)-partitioned: out
    [128=(d,i)chunk, 64 b] per kc-chunk: 2x fewer PE rows than v1.
  * d-reduction done ON PE via chained identity matmuls accumulating
    in PSUM (start/stop over d) - removes the DVE add tree entirely.
  * logits bl never materialized: e2 = e1 * exp(delta2) folds the
    cross-iteration accumulation into the exp chain.
  * softmax stays i-partitioned end-to-end: no DMA transposes.
  * squash in b-partitioned layout: a handful of [64, 10]-sized ops.
  * PSUM->SBUF conversion work (f32 G -> bf16 for 2x-mode DVE mult)
    is routed per-o across Act / DVE-direct / Pool to balance engines.
"""
import sys

sys.path.insert(0, "/opt/trn_rl_repo")

import numpy as np
import ml_dtypes
from contextlib import ExitStack

from concourse import bacc, mybir, hw_specs
from concourse.tile import TileContext
from concourse.bass_utils import run_bass_kernel_spmd

BF16 = mybir.dt.bfloat16
F32 = mybir.dt.float32
AX = mybir.AxisListType
ALU = mybir.AluOpType
ACTF = mybir.ActivationFunctionType
bfnp = ml_dtypes.bfloat16

B = 64
I = 1152
T = 9          # i-chunks of 128 (also the softmax "c" index)
O = 10
J = 16
D = 8
KC = T * D     # 72 k-chunks of 128 over flat k = d*I + i
EPS = 1e-06
N_CORES = 8

_cache = {}

# Route every activation through the one table set that has exp+ln+copy,
# so the ACT engine never reloads tables mid-kernel.
_KEEP_SET = "natural_log_exp_and_others"


def _patched_tables(arch):
    full = {k: set(v) for k, v in hw_specs.get_activation_tables(arch).items()}
    keep = full[_KEEP_SET]
    return {k: (v if k == _KEEP_SET else v - keep) for k, v in full.items()}


import os
if os.environ.get('ACT_PATCH', '1') == '1':
    bacc.get_activation_tables = _patched_tables

# per-o conversion route for the m2 ug-mult (GPSIMD cannot read PSUM,
# so conversions are Act or DVE only):
#   'a' = Act copies PSUM f32 -> SBUF bf16, DVE multiplies at 2x
#   'A' = Act copies, Pool multiplies (SBUF-only, legal)
#   'b' = DVE multiplies straight from PSUM f32 (1x, no Act work)
#   'm' = per-chunk alternation: even chunks Act-copy, odd chunks
#         DVE-direct; DVE strided 2x mult for the copied half
ROUTES = os.environ.get("M2_ROUTES", "MMMMMMMMMM")
# per-o d-sum engine: 'T' PE identity-matmul chain (psum f32)
#                     'v' DVE in-place bf16 fold tree | 'g' Pool fold tree
DS_ENG = os.environ.get("DS_ENG", "TgTTvTTgvT")
# per-(it,o) cu engine: 'v' DVE | 'g' Pool  (20 chars: it1 o0-9, it2 o0-9)
CU_ENG = os.environ.get("CU_ENG", "vgvvgvgvgv" "vgvvgvgvgv")
# engine for per-o c = e*rz mult: 'v' DVE | 'g' Pool
C_ENG = os.environ.get("C_ENG", "gv" * 10)
if len(C_ENG) == 1:
    C_ENG = C_ENG * 20
# per-o count of Act-copied chunk-groups in m/M routes (rest DVE-direct)
M_H = os.environ.get("M_H", "5555545545")
# engine for v_jb strip copies: 'a' Act | 'v' DVE
VJB_ENG = os.environ.get("VJB_ENG", "v")
# per-o engine for the pass-2 e = e*exp(delta) mult: 'v' DVE | 'g' Pool
E_ENG = os.environ.get("E_ENG", "vgvgvgvgvg")
SPLIT_MULT = os.environ.get("SPLIT_MULT", "1") == "1"
DVE_FIRST = os.environ.get("DVE_FIRST", "0") == "1"


def build_nc():
    nc = bacc.Bacc()
    ws_d = nc.dram_tensor("ws", [128, T, D, O, J], BF16, kind="ExternalInput")
    ui_d = nc.dram_tensor("ui", [128, T, D, B], BF16, kind="ExternalInput")
    ui2_d = nc.dram_tensor("ui2", [128, KC, B], BF16, kind="ExternalInput")
    wb2_d = nc.dram_tensor("wb2", [128, 3, KC, 128], BF16, kind="ExternalInput")
    id128_d = nc.dram_tensor("id128", [128, 128], BF16, kind="ExternalInput")
    id64_d = nc.dram_tensor("id64", [64, 64], BF16, kind="ExternalInput")
    vout_d = nc.dram_tensor("vout", [B, O, J], F32, kind="ExternalOutput")

    with TileContext(nc) as tc, ExitStack() as ctx:
        static = ctx.enter_context(tc.tile_pool(name="static", bufs=1))
        work = ctx.enter_context(tc.tile_pool(name="work", bufs=1))
        gsbp = ctx.enter_context(tc.tile_pool(name="gsbp", bufs=2))
        ugp = ctx.enter_context(tc.tile_pool(
            name="ugp", bufs=int(os.environ.get("DS_SHIFT", "3")) + 2))
        cup = ctx.enter_context(tc.tile_pool(name="cup", bufs=2))
        etp = ctx.enter_context(tc.tile_pool(name="etp", bufs=2))
        cop = ctx.enter_context(tc.tile_pool(name="cop", bufs=2))
        zp = ctx.enter_context(tc.tile_pool(name="zp", bufs=2))
        psS = ctx.enter_context(tc.tile_pool(name="psS", bufs=1, space="PSUM"))
        psVT = ctx.enter_context(tc.tile_pool(name="psVT", bufs=1, space="PSUM"))
        psG = ctx.enter_context(tc.tile_pool(
            name="psG", bufs=int(os.environ.get("PSG_BUFS", "4")), space="PSUM"))
        psDS = ctx.enter_context(tc.tile_pool(
            name="psDS", bufs=int(os.environ.get("PSDS_BUFS", "1")), space="PSUM"))

        # PE p-state: the clock ramps per continuous-busy stretch (reset on
        # idle; full speed only after 3us busy).  Dummy matmuls keep the PE
        # clock hot through DMA waits and phase boundaries.
        warm = static.tile([128, 128], BF16, name="warm")
        nc.vector.memset(warm, 0.0)

        def pe_keepalive(n):
            for _ in range(n):
                wps = psVT.tile([128, 96], F32, name="wps", tag="vt0")
                nc.tensor.matmul(wps, warm, warm[:, 0:96], start=True,
                                 stop=True, tile_position=(0, 0))

        pe_keepalive(int(os.environ.get("WARM0", "75")))

        ws = static.tile([128, T, D, O, J], BF16, name="ws")
        ui = static.tile([128, T, D, B], BF16, name="ui")
        ui2 = static.tile([128, KC, B], BF16, name="ui2")
        wb2 = static.tile([128, 3, KC, 128], BF16, name="wb2")
        id128 = static.tile([128, 128], BF16, name="id128")
        id64 = static.tile([64, 64], BF16, name="id64")
        eps1 = static.tile([64, 1], F32, name="eps1")
        nc.vector.memset(eps1, EPS)

        # DMA cost model (legacy CoreSim): each DMA holds the issuing
        # engine's queue for ~1.7us fixed + per-partition-free-bytes *
        # 0.39ns.  So: few big DMAs, spread across the SP / Act / Pool
        # queues, ordered by first use.
        # SP:   ws t0-4, id64, wb2 slot1, wb2 slot2
        # Pool: ws t5-8, wb2 slot0, id128
        # Act:  ui, ui2   (Act must be free for squash-0 at ~12us)
        if os.environ.get("DMA_PLAN", "A") == "A":
            nc.sync.dma_start(out=ws[:, 0:5], in_=ws_d[:, 0:5])
            nc.gpsimd.dma_start(out=ws[:, 5:9], in_=ws_d[:, 5:9])
            nc.scalar.dma_start(out=ui, in_=ui_d[:, :])
            nc.sync.dma_start(out=id64, in_=id64_d[:, :])
            nc.gpsimd.dma_start(out=wb2[:, 0], in_=wb2_d[:, 0])
            nc.gpsimd.dma_start(out=id128, in_=id128_d[:, :])
            nc.scalar.dma_start(out=ui2, in_=ui2_d[:, :])
            nc.sync.dma_start(out=wb2[:, 1], in_=wb2_d[:, 1])
            nc.sync.dma_start(out=wb2[:, 2], in_=wb2_d[:, 2])
        else:
            # plan E: per-t ws chunks alternating SP/Pool so the m1_A chain
            # streams at DMA pitch; slot0 split across both queues after.
            for t in range(0, 9, 2):
                nc.sync.dma_start(out=ws[:, t], in_=ws_d[:, t])
            for t in range(1, 9, 2):
                nc.gpsimd.dma_start(out=ws[:, t], in_=ws_d[:, t])
            nc.scalar.dma_start(out=ui, in_=ui_d[:, :])
            nc.scalar.dma_start(out=id64, in_=id64_d[:, :])
            nc.sync.dma_start(out=wb2[:, 0, 0:36], in_=wb2_d[:, 0, 0:36])
            nc.gpsimd.dma_start(out=wb2[:, 0, 36:72], in_=wb2_d[:, 0, 36:72])
            nc.scalar.dma_start(out=ui2, in_=ui2_d[:, :])
            nc.sync.dma_start(out=id128, in_=id128_d[:, :])
            nc.sync.dma_start(out=wb2[:, 1], in_=wb2_d[:, 1])
            nc.gpsimd.dma_start(out=wb2[:, 2], in_=wb2_d[:, 2])

        # persistent work tiles
        e = work.tile([128, O, T, B], BF16, name="e")
        rz_f = work.tile([128, T, B], F32, name="rz_f")
        rzb = work.tile([128, T, B], BF16, name="rzb")
        v_f = work.tile([64, O, J], F32, name="v_f")
        v_b = work.tile([64, O, J], BF16, name="v_b")
        v_jb = work.tile([128, 3, B], BF16, name="v_jb")
        s2 = work.tile([64, O, J], F32, name="s2")
        sq = work.tile([64, O], F32, name="sq")
        t1 = work.tile([64, O], F32, name="t1")
        den = work.tile([64, O], F32, name="den")
        rcp = work.tile([64, O], F32, name="rcp")
        ff = work.tile([64, O], F32, name="ff")

        s_ps = psS.tile([64, O, J], F32, name="s_ps")

        def m1_A_chain(h):
            """it0 half h: c uniform -> s_raw[b, o-half] = sum_{i,d} W u."""
            o5 = slice(5 * h, 5 * h + 5)
            for t in range(T):
                for d in range(D):
                    td = t * D + d
                    nc.tensor.matmul(
                        s_ps[:, o5, :].rearrange("p o j -> p (o j)"),
                        ui[:, t, d, :],
                        ws[:, t, d, o5, :].rearrange("p o j -> p (o j)"),
                        start=(td == 0), stop=(td == KC - 1),
                        tile_position=(0, 0), skip_group_check=True,
                    )

        def squash(it, h):
            """v[:, half] = squash(scale * s_ps[:, half]), tiny b-part ops."""
            scale = 0.1 if it == 0 else 1.0
            o5 = slice(5 * h, 5 * h + 5)
            nc.scalar.activation(s2[:, o5, :], s_ps[:, o5, :], ACTF.Square,
                                 scale=scale)
            nc.vector.tensor_reduce(sq[:, o5], s2[:, o5, :], axis=AX.X,
                                    op=ALU.add)
            nc.scalar.activation(t1[:, o5], sq[:, o5], ACTF.Ln, bias=eps1)
            nc.scalar.activation(den[:, o5], t1[:, o5], ACTF.Exp, scale=0.5)
            nc.vector.tensor_scalar_add(t1[:, o5], sq[:, o5], 1.0)
            nc.vector.tensor_tensor(den[:, o5], den[:, o5], t1[:, o5],
                                    op=ALU.mult)
            nc.vector.reciprocal(rcp[:, o5], den[:, o5])
            nc.vector.tensor_tensor(ff[:, o5], sq[:, o5], rcp[:, o5],
                                    op=ALU.mult)
            if it == 0:
                nc.vector.tensor_scalar_mul(ff[:, o5], ff[:, o5], scale)
            nc.vector.tensor_tensor(
                v_f[:, o5, :], s_ps[:, o5, :],
                ff[:, o5].unsqueeze(2).broadcast_to([64, 5, J]),
                op=ALU.mult)
            nc.vector.tensor_copy(
                v_b[:, o5, :].rearrange("p o j -> p (o j)"),
                v_f[:, o5, :].rearrange("p o j -> p (o j)"))

        # half-h (g, sl) slots are disjoint: h0 -> sl0 strips + (g0, sl1);
        # h1 -> (g1..3, sl1) + (g0..1, sl2).
        def transposes(h):
            vt = psVT.tile([128, 3, B], BF16, name="vt", tag="vt0",
                           bufs=1)
            for o in range(5 * h, 5 * h + 5):
                g, sl = o % 4, o // 4
                nc.tensor.matmul(
                    vt[32 * g : 32 * g + 16, sl, :],
                    v_b[:, o, :], id64,
                    is_transpose=True, tile_position=(0, 32 * g),
                )
            for o in range(5 * h, 5 * h + 5):
                g, sl = o % 4, o // 4
                if VJB_ENG == "a":
                    nc.scalar.copy(v_jb[32 * g : 32 * g + 16, sl, :],
                                   vt[32 * g : 32 * g + 16, sl, :])
                else:
                    nc.vector.tensor_copy(
                        v_jb[32 * g : 32 * g + 16, sl, :],
                        vt[32 * g : 32 * g + 16, sl, :])

        flat = lambda ap: ap.rearrange("p t b -> p (t b)")
        flat3 = lambda ap: ap.rearrange("p a b -> p (a b)")

        def emit_G(o, route):
            """G^T chunks for o; returns the ug tile being filled."""
            g, sl = o % 4, o // 4
            ug = ugp.tile([128, KC, B], BF16, name="ug", tag="ug")
            gsb = None
            if route != "b":
                gsb = gsbp.tile([128, KC, B], BF16, name="gsb", tag="gsb")

            def gmm(pg, kk, kc):
                nc.tensor.matmul(
                    pg[:, kk, :],
                    wb2[32 * g : 32 * g + 16, sl, kc, :],
                    v_jb[32 * g : 32 * g + 16, sl, :],
                    start=True, stop=True,
                    tile_position=(32 * g, 0),
                )

            if route == "6":
                # 16-kc psG tiles: (Act, Act, DVE, DVE, Act-half); Pool
                # multiplies the Act-copied parts.
                for ti in range(5):
                    k0 = 16 * ti
                    nk = 16 if ti < 4 else 8
                    pg = psG.tile([128, 16, B], F32, name="pg", tag="pg")
                    for kk in range(nk):
                        gmm(pg, kk, k0 + kk)
                    slk = slice(k0, k0 + nk)
                    if ti in (0, 1, 4):
                        nc.scalar.copy(flat3(gsb[:, slk, :]),
                                       flat3(pg[:, 0:nk, :]))
                    else:
                        nc.vector.tensor_tensor(
                            flat3(ug[:, slk, :]), flat3(pg[:, 0:nk, :]),
                            flat3(ui2[:, slk, :]), op=ALU.mult)
                nc.gpsimd.tensor_tensor(
                    flat3(ug[:, 0:32, :]), flat3(gsb[:, 0:32, :]),
                    flat3(ui2[:, 0:32, :]), op=ALU.mult)
                nc.gpsimd.tensor_tensor(
                    flat3(ug[:, 64:72, :]), flat3(gsb[:, 64:72, :]),
                    flat3(ui2[:, 64:72, :]), op=ALU.mult)
                return ug

            nA = int(M_H[o]) if route in ("m", "M") else 9
            horder = list(range(9))
            if DVE_FIRST and route in ("m", "M"):
                horder = list(range(nA, 9)) + list(range(nA))
            for h in horder:
                pg = psG.tile([128, 8, B], F32, name="pg", tag="pg")
                for kk in range(8):
                    gmm(pg, kk, 8 * h + kk)
                sl8 = slice(8 * h, 8 * h + 8)
                if route in ("a", "A") or (route in ("m", "M") and h < nA):
                    nc.scalar.copy(flat3(gsb[:, sl8, :]),
                                   flat3(pg[:, 0:8, :]))
                else:  # DVE straight from PSUM
                    nc.vector.tensor_tensor(
                        flat3(ug[:, sl8, :]), flat3(pg[:, 0:8, :]),
                        flat3(ui2[:, sl8, :]), op=ALU.mult)
            if route in ("a", "A"):
                meng = nc.gpsimd if route == "A" else nc.vector
                meng.tensor_tensor(flat3(ug), flat3(gsb), flat3(ui2),
                                   op=ALU.mult)
            elif route in ("m", "M"):
                meng = nc.gpsimd if route == "M" else nc.vector
                if SPLIT_MULT and nA >= 3:
                    # two halves so the first can run while the later Act
                    # copies are still in flight
                    cut = 8 * (nA // 2 + 1)
                    for sl_ in (slice(0, cut), slice(cut, 8 * nA)):
                        meng.tensor_tensor(
                            flat3(ug[:, sl_, :]), flat3(gsb[:, sl_, :]),
                            flat3(ui2[:, sl_, :]), op=ALU.mult)
                else:
                    hA = slice(0, 8 * nA)
                    meng.tensor_tensor(
                        flat3(ug[:, hA, :]), flat3(gsb[:, hA, :]),
                        flat3(ui2[:, hA, :]), op=ALU.mult)
            return ug

        def emit_ds(o, ug):
            """delta[o] = sum_d ug chunks.  Returns (psum_tile|None, ug)."""
            eng = DS_ENG[o]
            if eng == "T":  # PE identity-matmul chains into PSUM f32
                ds = psDS.tile([128, T, B], F32, name="ds", tag="ds")
                for d in range(D):
                    nc.tensor.matmul(
                        flat3(ds[:, 0:8, :]), id128,
                        flat3(ug[:, d * T : d * T + 8, :]),
                        start=(d == 0), stop=(d == D - 1),
                        tile_position=(0, 0), skip_group_check=True,
                    )
                for d in range(D):
                    nc.tensor.matmul(
                        ds[:, 8, :], id128, ug[:, d * T + 8, :],
                        start=(d == 0), stop=(d == D - 1),
                        tile_position=(0, 0), skip_group_check=True,
                    )
                return ds, ug
            ve = nc.vector if eng == "v" else nc.gpsimd
            # in-place bf16 fold tree: 72 -> 36 -> 18 -> 9 chunks
            for w in (36, 18, 9):
                ve.tensor_tensor(flat3(ug[:, 0:w, :]), flat3(ug[:, 0:w, :]),
                                 flat3(ug[:, w : 2 * w, :]), op=ALU.add)
            return None, ug

        def emit_exp(o, dsug, r):
            ds, ug = dsug
            src = flat3(ds) if ds is not None else flat3(ug[:, 0:T, :])
            if r == 0:
                nc.scalar.activation(flat(e[:, o]), src, ACTF.Exp)
            else:
                et = etp.tile([128, T, B], BF16, name="et", tag="et")
                nc.scalar.activation(flat(et), src, ACTF.Exp)
                eeng = nc.gpsimd if E_ENG[o] == "g" else nc.vector
                eeng.tensor_tensor(flat(e[:, o]), flat(e[:, o]),
                                   flat(et), op=ALU.mult)

        def emit_zpair(q):
            """partial softmax sums on Pool, overlapped with m2."""
            zq = zp.tile([128, T, B], BF16, name="zq", tag=f"z{q}", bufs=1)
            nc.gpsimd.tensor_tensor(flat(zq), flat(e[:, 2 * q]),
                                    flat(e[:, 2 * q + 1]), op=ALU.add)
            _zpart.append(zq)
            if q in (1, 3):  # fold pairs into quads as soon as available
                zz = zp.tile([128, T, B], BF16, name="zz", tag=f"zz{q}",
                             bufs=1)
                nc.gpsimd.tensor_tensor(flat(zz), flat(_zpart[-2]),
                                        flat(_zpart[-1]), op=ALU.add)
                _zquad.append(zz)

        DS_SHIFT = int(os.environ.get("DS_SHIFT", "3"))

        def m2(r, it):
            """delta_o for all o -> e (pass r), software-pipelined.
            Caller has emitted squash(it,0)+transposes(0); squash/transposes
            of the second half are interleaved after G(1)."""
            ugs = {}
            dss = {}
            for step in range(O + DS_SHIFT + 1):
                if step < O:
                    ugs[step] = emit_G(step, ROUTES[step])
                if step == 1:
                    squash(it, 1)
                    transposes(1)
                if 0 <= step - DS_SHIFT < O:
                    dss[step - DS_SHIFT] = emit_ds(
                        step - DS_SHIFT, ugs.pop(step - DS_SHIFT))
                if 0 <= step - DS_SHIFT - 1 < O:
                    oo = step - DS_SHIFT - 1
                    emit_exp(oo, dss.pop(oo), r)
                    if oo % 2 == 1:
                        emit_zpair(oo // 2)
                if step == 3 and len(_zpart) >= 2:
                    pass

        def softmax_tail():
            """finish Z = sum_o e; rz = 1/Z (bf16)."""
            za = zp.tile([128, T, B], BF16, name="za", tag="za")
            nc.vector.tensor_tensor(flat(za), flat(_zquad[0]),
                                    flat(_zquad[1]), op=ALU.add)
            nc.vector.tensor_tensor(flat(za), flat(za), flat(_zpart[4]),
                                    op=ALU.add)
            with nc.allow_low_precision("softmax normalizer, 2e-2 tolerance"):
                nc.vector.reciprocal(flat(rzb), flat(za))
            _zpart.clear()
            _zquad.clear()

        def m1_B(it):
            """s[b, o, j] = sum_{i,d} (c_o u) W for all o.
            squash/transposes of half 0 are emitted after o=4's chain."""
            for o in range(O):
                co = cop.tile([128, T, B], BF16, name="co", tag="co")
                ceng = nc.gpsimd if C_ENG[(it - 1) * O + o] == "g" else nc.vector
                ceng.tensor_tensor(flat(co), flat(e[:, o]), flat(rzb),
                                   op=ALU.mult)
                cu = cup.tile([128, T, D, B], BF16, name="cu", tag="cu")
                cueng = nc.gpsimd if CU_ENG[(it - 1) * O + o] == "g" else nc.vector
                cueng.tensor_tensor(
                    cu[:, :, :, :],
                    co.unsqueeze(2).broadcast_to([128, T, D, B]),
                    ui[:, :, :, :], op=ALU.mult)
                for t in range(T):
                    for d in range(D):
                        td = t * D + d
                        nc.tensor.matmul(
                            s_ps[:, o, :], cu[:, t, d, :], ws[:, t, d, o, :],
                            start=(td == 0), stop=(td == KC - 1),
                            tile_position=(0, 0), skip_group_check=True,
                        )
                if o == 4:
                    squash(it, 0)
                    if it < 2:
                        transposes(0)
                    else:
                        nc.sync.dma_start(out=vout_d[:, 0:5, :],
                                          in_=v_f[:, 0:5, :])

        _zpart = []
        _zquad = []

        # ========================= flow =========================
        W1 = int(os.environ.get("WARM1", "0"))
        W2 = int(os.environ.get("WARM2", "0"))
        m1_A_chain(0)
        squash(0, 0)
        m1_A_chain(1)
        transposes(0)
        for r in range(2):
            m2(r, r)
            pe_keepalive(W1)
            softmax_tail()
            m1_B(r + 1)
            pe_keepalive(W2)
        squash(2, 1)
        nc.sync.dma_start(out=vout_d[:, 5:10, :], in_=v_f[:, 5:10, :])

    nc.finalize()
    return nc


def _host_prep(u, weights):
    """Per-core input maps. u [512,1152,8] f32, weights [1152,10,16,8] f32."""
    W = np.asarray(weights, dtype=np.float32)
    u = np.asarray(u, dtype=np.float32)
    # ws[p, t, d, o, j] = W[t*128+p, o, j, d]
    ws = np.ascontiguousarray(
        W.reshape(T, 128, O, J, D).transpose(1, 0, 4, 2, 3)
    ).astype(bfnp)
    # wb2[32g+jj, sl, kc, m] = W[c*128+m, o, jj, d], kc = d*T + c
    wt = W.reshape(T, 128, O, J, D)  # [c, m, o, j, d]
    wb2 = np.zeros((128, 3, KC, 128), dtype=bfnp)
    for o in range(O):
        g, sl = o % 4, o // 4
        blk = wt[:, :, o, :, :].transpose(2, 3, 0, 1)  # [j, d, c, m]
        wb2[32 * g : 32 * g + 16, sl] = blk.reshape(J, KC, 128).astype(bfnp)
    id128 = np.eye(128, dtype=np.float32).astype(bfnp)
    id64 = np.eye(64, dtype=np.float32).astype(bfnp)

    base = {"ws": ws, "wb2": wb2, "id128": id128, "id64": id64}
    in_maps = []
    for c in range(N_CORES):
        uc = u[c * B : (c + 1) * B]  # [64, 1152, 8]
        ur = uc.reshape(B, T, 128, D)
        ui = np.ascontiguousarray(ur.transpose(2, 1, 3, 0)).astype(bfnp)
        # ui2[p, kc, b] = u[b, c*128+p, d], kc = d*T + c
        ui2 = np.ascontiguousarray(
            ur.transpose(2, 3, 1, 0).reshape(128, D * T, B)
        ).astype(bfnp)
        in_maps.append({**base, "ui": ui, "ui2": ui2})
    return in_maps


def kernel(u, weights):
    if "nc" not in _cache:
        _cache["nc"] = build_nc()
    nc = _cache["nc"]
    in_maps = _host_prep(u, weights)
    res = run_bass_kernel_spmd(nc, in_maps, core_ids=list(range(N_CORES)))
    out = np.concatenate([res.results[c]["vout"] for c in range(N_CORES)], axis=0)
    return out.astype(np.float32)


if __name__ == "__main__":
    rng = np.random.default_rng(0)
    u = rng.standard_normal((512, 1152, 8), dtype=np.float32)
    w = (rng.standard_normal((1152, 10, 16, 8)) * 0.1).astype(np.float32)
    v = kernel(u, w)
    print("out", v.shape, v.dtype, np.abs(v).max())


# revision 9
# speedup vs baseline: 1.0022x; 1.0022x over previous
"""CapsNet dynamic-routing layer on 8 Trainium2 NeuronCores (Bass/Tile), v2.

reference math (per batch element b):
  u_hat[b,i,o,j] = sum_d W[i,o,j,d] * u[b,i,d]        (never materialized)
  bl = 0; for r in 0..2:
    c = softmax_o(bl); s[b,o,j] = sum_i c*u_hat; v = squash(s)
    if r < 2: bl += sum_j u_hat*v
  return v  [B, 10, 16]

Distribution: pure data parallel, batch 512 -> 64 per core x 8 cores;
weights replicated.  Per-core: b=64, i=1152=9*128, o=10, j=16, d=8.

v2 design vs v1:
  * m1 (s-matmuls) output b-partitioned [64, (o,j)] -> n=16/o per
    instruction instead of n=64: 4x fewer PE rows.
  * m2 (agreement) G^T = W^T v computed (d,i)-partitioned: out
    [128=(d,i)chunk, 64 b] per kc-chunk: 2x fewer PE rows than v1.
  * d-reduction done ON PE via chained identity matmuls accumulating
    in PSUM (start/stop over d) - removes the DVE add tree entirely.
  * logits bl never materialized: e2 = e1 * exp(delta2) folds the
    cross-iteration accumulation into the exp chain.
  * softmax stays i-partitioned end-to-end: no DMA transposes.
  * squash in b-partitioned layout: a handful of [64, 10]-sized ops.
  * PSUM->SBUF conversion work (f32 G -> bf16 for 2x-mode DVE mult)
    is routed per-o across Act / DVE-direct / Pool to balance engines.
"""
import sys

sys.path.insert(0, "/opt/trn_rl_repo")

import numpy as np
import ml_dtypes
from contextlib import ExitStack

from concourse import bacc, mybir, hw_specs
from concourse.tile import TileContext
from concourse.bass_utils import run_bass_kernel_spmd

BF16 = mybir.dt.bfloat16
F32 = mybir.dt.float32
AX = mybir.AxisListType
ALU = mybir.AluOpType
ACTF = mybir.ActivationFunctionType
bfnp = ml_dtypes.bfloat16

B = 64
I = 1152
T = 9          # i-chunks of 128 (also the softmax "c" index)
O = 10
J = 16
D = 8
KC = T * D     # 72 k-chunks of 128 over flat k = d*I + i
EPS = 1e-06
N_CORES = 8

_cache = {}

# Route every activation through the one table set that has exp+ln+copy,
# so the ACT engine never reloads tables mid-kernel.
_KEEP_SET = "natural_log_exp_and_others"


def _patched_tables(arch):
    full = {k: set(v) for k, v in hw_specs.get_activation_tables(arch).items()}
    keep = full[_KEEP_SET]
    return {k: (v if k == _KEEP_SET else v - keep) for k, v in full.items()}


import os
if os.environ.get('ACT_PATCH', '1') == '1':
    bacc.get_activation_tables = _patched_tables

# per-o conversion route for the m2 ug-mult (GPSIMD cannot read PSUM,
# so conversions are Act or DVE only):
#   'a' = Act copies PSUM f32 -> SBUF bf16, DVE multiplies at 2x
#   'A' = Act copies, Pool multiplies (SBUF-only, legal)
#   'b' = DVE multiplies straight from PSUM f32 (1x, no Act work)
#   'm' = per-chunk alternation: even chunks Act-copy, odd chunks
#         DVE-direct; DVE strided 2x mult for the copied half
ROUTES = os.environ.get("M2_ROUTES", "MMMMMMMMMM")
# per-o d-sum engine: 'T' PE identity-matmul chain (psum f32)
#                     'v' DVE in-place bf16 fold tree | 'g' Pool fold tree
DS_ENG = os.environ.get("DS_ENG", "TgTTvTTgvT")
# per-(it,o) cu engine: 'v' DVE | 'g' Pool  (20 chars: it1 o0-9, it2 o0-9)
CU_ENG = os.environ.get("CU_ENG", "vgvvgvgvgv" "vgvvgvgvgv")
# engine for per-o c = e*rz mult: 'v' DVE | 'g' Pool
C_ENG = os.environ.get("C_ENG", "gv" * 10)
if len(C_ENG) == 1:
    C_ENG = C_ENG * 20
# per-o count of Act-copied chunk-groups in m/M routes (rest DVE-direct)
M_H = os.environ.get("M_H", "5555545545")
# engine for v_jb strip copies: 'a' Act | 'v' DVE
VJB_ENG = os.environ.get("VJB_ENG", "v")
# per-o engine for the pass-2 e = e*exp(delta) mult: 'v' DVE | 'g' Pool
E_ENG = os.environ.get("E_ENG", "vgvgvgvgvg")
SPLIT_MULT = os.environ.get("SPLIT_MULT", "1") == "1"
DVE_FIRST = os.environ.get("DVE_FIRST", "0") == "1"


def build_nc():
    nc = bacc.Bacc()
    ws_d = nc.dram_tensor("ws", [128, T, D, O, J], BF16, kind="ExternalInput")
    ui_d = nc.dram_tensor("ui", [128, T, D, B], BF16, kind="ExternalInput")
    ui2_d = nc.dram_tensor("ui2", [128, KC, B], BF16, kind="ExternalInput")
    wb2_d = nc.dram_tensor("wb2", [128, 3, KC, 128], BF16, kind="ExternalInput")
    id128_d = nc.dram_tensor("id128", [128, 128], BF16, kind="ExternalInput")
    id64_d = nc.dram_tensor("id64", [64, 64], BF16, kind="ExternalInput")
    vout_d = nc.dram_tensor("vout", [B, O, J], F32, kind="ExternalOutput")

    with TileContext(nc) as tc, ExitStack() as ctx:
        static = ctx.enter_context(tc.tile_pool(name="static", bufs=1))
        work = ctx.enter_context(tc.tile_pool(name="work", bufs=1))
        gsbp = ctx.enter_context(tc.tile_pool(name="gsbp", bufs=2))
        ugp = ctx.enter_context(tc.tile_pool(
            name="ugp", bufs=int(os.environ.get("DS_SHIFT", "3")) + 2))
        cup = ctx.enter_context(tc.tile_pool(name="cup", bufs=2))
        etp = ctx.enter_context(tc.tile_pool(name="etp", bufs=2))
        cop = ctx.enter_context(tc.tile_pool(name="cop", bufs=2))
        zp = ctx.enter_context(tc.tile_pool(name="zp", bufs=2))
        psS = ctx.enter_context(tc.tile_pool(name="psS", bufs=1, space="PSUM"))
        psVT = ctx.enter_context(tc.tile_pool(name="psVT", bufs=1, space="PSUM"))
        psG = ctx.enter_context(tc.tile_pool(
            name="psG", bufs=int(os.environ.get("PSG_BUFS", "4")), space="PSUM"))
        psDS = ctx.enter_context(tc.tile_pool(
            name="psDS", bufs=int(os.environ.get("PSDS_BUFS", "1")), space="PSUM"))

        # PE p-state: the clock ramps per continuous-busy stretch (reset on
        # idle; full speed only after 3us busy).  Dummy matmuls keep the PE
        # clock hot through DMA waits and phase boundaries.
        warm = static.tile([128, 128], BF16, name="warm")
        nc.vector.memset(warm, 0.0)

        def pe_keepalive(n):
            for _ in range(n):
                wps = psVT.tile([128, 96], F32, name="wps", tag="vt0")
                nc.tensor.matmul(wps, warm, warm[:, 0:96], start=True,
                                 stop=True, tile_position=(0, 0))

        pe_keepalive(int(os.environ.get("WARM0", "75")))

        ws = static.tile([128, T, D, O, J], BF16, name="ws")
        ui = static.tile([128, T, D, B], BF16, name="ui")
        ui2 = static.tile([128, KC, B], BF16, name="ui2")
        wb2 = static.tile([128, 3, KC, 128], BF16, name="wb2")
        id128 = static.tile([128, 128], BF16, name="id128")
        id64 = static.tile([64, 64], BF16, name="id64")
        eps1 = static.tile([64, 1], F32, name="eps1")
        nc.vector.memset(eps1, EPS)

        # DMA cost model (legacy CoreSim): each DMA holds the issuing
        # engine's queue for ~1.7us fixed + per-partition-free-bytes *
        # 0.39ns.  So: few big DMAs, spread across the SP / Act / Pool
        # queues, ordered by first use.
        # SP:   ws t0-4, id64, wb2 slot1, wb2 slot2
        # Pool: ws t5-8, wb2 slot0, id128
        # Act:  ui, ui2   (Act must be free for squash-0 at ~12us)
        if os.environ.get("DMA_PLAN", "A") == "A":
            nc.sync.dma_start(out=ws[:, 0:5], in_=ws_d[:, 0:5])
            nc.gpsimd.dma_start(out=ws[:, 5:9], in_=ws_d[:, 5:9])
            nc.scalar.dma_start(out=ui, in_=ui_d[:, :])
            nc.sync.dma_start(out=id64, in_=id64_d[:, :])
            nc.gpsimd.dma_start(out=wb2[:, 0], in_=wb2_d[:, 0])
            nc.gpsimd.dma_start(out=id128, in_=id128_d[:, :])
            nc.scalar.dma_start(out=ui2, in_=ui2_d[:, :])
            nc.sync.dma_start(out=wb2[:, 1], in_=wb2_d[:, 1])
            nc.sync.dma_start(out=wb2[:, 2], in_=wb2_d[:, 2])
        else:
            # plan E: per-t ws chunks alternating SP/Pool so the m1_A chain
            # streams at DMA pitch; slot0 split across both queues after.
            for t in range(0, 9, 2):
                nc.sync.dma_start(out=ws[:, t], in_=ws_d[:, t])
            for t in range(1, 9, 2):
                nc.gpsimd.dma_start(out=ws[:, t], in_=ws_d[:, t])
            nc.scalar.dma_start(out=ui, in_=ui_d[:, :])
            nc.scalar.dma_start(out=id64, in_=id64_d[:, :])
            nc.sync.dma_start(out=wb2[:, 0, 0:36], in_=wb2_d[:, 0, 0:36])
            nc.gpsimd.dma_start(out=wb2[:, 0, 36:72], in_=wb2_d[:, 0, 36:72])
            nc.scalar.dma_start(out=ui2, in_=ui2_d[:, :])
            nc.sync.dma_start(out=id128, in_=id128_d[:, :])
            nc.sync.dma_start(out=wb2[:, 1], in_=wb2_d[:, 1])
            nc.gpsimd.dma_start(out=wb2[:, 2], in_=wb2_d[:, 2])

        # persistent work tiles
        e = work.tile([128, O, T, B], BF16, name="e")
        rz_f = work.tile([128, T, B], F32, name="rz_f")
        rzb = work.tile([128, T, B], BF16, name="rzb")
        v_f = work.tile([64, O, J], F32, name="v_f")
        v_b = work.tile([64, O, J], BF16, name="v_b")
        v_jb = work.tile([128, 3, B], BF16, name="v_jb")
        s2 = work.tile([64, O, J], F32, name="s2")
        sq = work.tile([64, O], F32, name="sq")
        t1 = work.tile([64, O], F32, name="t1")
        den = work.tile([64, O], F32, name="den")
        rcp = work.tile([64, O], F32, name="rcp")
        ff = work.tile([64, O], F32, name="ff")

        s_ps = psS.tile([64, O, J], F32, name="s_ps")

        def m1_A_chain(h):
            """it0 half h: c uniform -> s_raw[b, o-half] = sum_{i,d} W u."""
            o5 = slice(5 * h, 5 * h + 5)
            for t in range(T):
                for d in range(D):
                    td = t * D + d
                    nc.tensor.matmul(
                        s_ps[:, o5, :].rearrange("p o j -> p (o j)"),
                        ui[:, t, d, :],
                        ws[:, t, d, o5, :].rearrange("p o j -> p (o j)"),
                        start=(td == 0), stop=(td == KC - 1),
                        tile_position=(0, 0), skip_group_check=True,
                    )

        def squash(it, h):
            """v[:, half] = squash(scale * s_ps[:, half]), tiny b-part ops."""
            scale = 0.1 if it == 0 else 1.0
            o5 = slice(5 * h, 5 * h + 5)
            nc.scalar.activation(s2[:, o5, :], s_ps[:, o5, :], ACTF.Square,
                                 scale=scale)
            nc.vector.tensor_reduce(sq[:, o5], s2[:, o5, :], axis=AX.X,
                                    op=ALU.add)
            nc.scalar.activation(t1[:, o5], sq[:, o5], ACTF.Ln, bias=eps1)
            nc.scalar.activation(den[:, o5], t1[:, o5], ACTF.Exp, scale=0.5)
            nc.vector.tensor_scalar_add(t1[:, o5], sq[:, o5], 1.0)
            nc.vector.tensor_tensor(den[:, o5], den[:, o5], t1[:, o5],
                                    op=ALU.mult)
            nc.vector.reciprocal(rcp[:, o5], den[:, o5])
            nc.vector.tensor_tensor(ff[:, o5], sq[:, o5], rcp[:, o5],
                                    op=ALU.mult)
            if it == 0:
                nc.vector.tensor_scalar_mul(ff[:, o5], ff[:, o5], scale)
            nc.vector.tensor_tensor(
                v_f[:, o5, :], s_ps[:, o5, :],
                ff[:, o5].unsqueeze(2).broadcast_to([64, 5, J]),
                op=ALU.mult)
            nc.vector.tensor_copy(
                v_b[:, o5, :].rearrange("p o j -> p (o j)"),
                v_f[:, o5, :].rearrange("p o j -> p (o j)"))

        # half-h (g, sl) slots are disjoint: h0 -> sl0 strips + (g0, sl1);
        # h1 -> (g1..3, sl1) + (g0..1, sl2).
        def transposes(h):
            vt = psVT.tile([128, 3, B], BF16, name="vt", tag="vt0",
                           bufs=1)
            for o in range(5 * h, 5 * h + 5):
                g, sl = o % 4, o // 4
                nc.tensor.matmul(
                    vt[32 * g : 32 * g + 16, sl, :],
                    v_b[:, o, :], id64,
                    is_transpose=True, tile_position=(0, 32 * g),
                )
            for o in range(5 * h, 5 * h + 5):
                g, sl = o % 4, o // 4
                if VJB_ENG == "a":
                    nc.scalar.copy(v_jb[32 * g : 32 * g + 16, sl, :],
                                   vt[32 * g : 32 * g + 16, sl, :])
                else:
                    nc.vector.tensor_copy(
                        v_jb[32 * g : 32 * g + 16, sl, :],
                        vt[32 * g : 32 * g + 16, sl, :])

        flat = lambda ap: ap.rearrange("p t b -> p (t b)")
        flat3 = lambda ap: ap.rearrange("p a b -> p (a b)")

        def emit_G(o, route):
            """G^T chunks for o; returns the ug tile being filled."""
            g, sl = o % 4, o // 4
            ug = ugp.tile([128, KC, B], BF16, name="ug", tag="ug")
            gsb = None
            if route != "b":
                gsb = gsbp.tile([128, KC, B], BF16, name="gsb", tag="gsb")

            def gmm(pg, kk, kc):
                nc.tensor.matmul(
                    pg[:, kk, :],
                    wb2[32 * g : 32 * g + 16, sl, kc, :],
                    v_jb[32 * g : 32 * g + 16, sl, :],
                    start=True, stop=True,
                    tile_position=(32 * g, 0),
                )

            if route == "6":
                # 16-kc psG tiles: (Act, Act, DVE, DVE, Act-half); Pool
                # multiplies the Act-copied parts.
                for ti in range(5):
                    k0 = 16 * ti
                    nk = 16 if ti < 4 else 8
                    pg = psG.tile([128, 16, B], F32, name="pg", tag="pg")
                    for kk in range(nk):
                        gmm(pg, kk, k0 + kk)
                    slk = slice(k0, k0 + nk)
                    if ti in (0, 1, 4):
                        nc.scalar.copy(flat3(gsb[:, slk, :]),
                                       flat3(pg[:, 0:nk, :]))
                    else:
                        nc.vector.tensor_tensor(
                            flat3(ug[:, slk, :]), flat3(pg[:, 0:nk, :]),
                            flat3(ui2[:, slk, :]), op=ALU.mult)
                nc.gpsimd.tensor_tensor(
                    flat3(ug[:, 0:32, :]), flat3(gsb[:, 0:32, :]),
                    flat3(ui2[:, 0:32, :]), op=ALU.mult)
                nc.gpsimd.tensor_tensor(
                    flat3(ug[:, 64:72, :]), flat3(gsb[:, 64:72, :]),
                    flat3(ui2[:, 64:72, :]), op=ALU.mult)
                return ug

            nA = int(M_H[o]) if route in ("m", "M") else 9
            horder = list(range(9))
            if DVE_FIRST and route in ("m", "M"):
                horder = list(range(nA, 9)) + list(range(nA))
            for h in horder:
                pg = psG.tile([128, 8, B], F32, name="pg", tag="pg")
                for kk in range(8):
                    gmm(pg, kk, 8 * h + kk)
                sl8 = slice(8 * h, 8 * h + 8)
                if route in ("a", "A") or (route in ("m", "M") and h < nA):
                    nc.scalar.copy(flat3(gsb[:, sl8, :]),
                                   flat3(pg[:, 0:8, :]))
                else:  # DVE straight from PSUM
                    nc.vector.tensor_tensor(
                        flat3(ug[:, sl8, :]), flat3(pg[:, 0:8, :]),
                        flat3(ui2[:, sl8, :]), op=ALU.mult)
            if route in ("a", "A"):
                meng = nc.gpsimd if route == "A" else nc.vector
                meng.tensor_tensor(flat3(ug), flat3(gsb), flat3(ui2),
                                   op=ALU.mult)
            elif route in ("m", "M"):
                meng = nc.gpsimd if route == "M" else nc.vector
                if SPLIT_MULT and nA >= 3:
                    # two halves so the first can run while the later Act
                    # copies are still in flight
                    cut = 8 * (nA // 2 + 1)
                    for sl_ in (slice(0, cut), slice(cut, 8 * nA)):
                        meng.tensor_tensor(
                            flat3(ug[:, sl_, :]), flat3(gsb[:, sl_, :]),
                            flat3(ui2[:, sl_, :]), op=ALU.mult)
                else:
                    hA = slice(0, 8 * nA)
                    meng.tensor_tensor(
                        flat3(ug[:, hA, :]), flat3(gsb[:, hA, :]),
                        flat3(ui2[:, hA, :]), op=ALU.mult)
            return ug

        def emit_ds(o, ug):
            """delta[o] = sum_d ug chunks.  Returns (psum_tile|None, ug)."""
            eng = DS_ENG[o]
            if eng == "T":  # PE identity-matmul chains into PSUM f32
                ds = psDS.tile([128, T, B], F32, name="ds", tag="ds")
                for d in range(D):
                    nc.tensor.matmul(
                        flat3(ds[:, 0:8, :]), id128,
                        flat3(ug[:, d * T : d * T + 8, :]),
                        start=(d == 0), stop=(d == D - 1),
                        tile_position=(0, 0), skip_group_check=True,
                    )
                for d in range(D):
                    nc.tensor.matmul(
                        ds[:, 8, :], id128, ug[:, d * T + 8, :],
                        start=(d == 0), stop=(d == D - 1),
                        tile_position=(0, 0), skip_group_check=True,
                    )
                return ds, ug
            ve = nc.vector if eng == "v" else nc.gpsimd
            # in-place bf16 fold tree: 72 -> 36 -> 18 -> 9 chunks
            for w in (36, 18, 9):
                ve.tensor_tensor(flat3(ug[:, 0:w, :]), flat3(ug[:, 0:w, :]),
                                 flat3(ug[:, w : 2 * w, :]), op=ALU.add)
            return None, ug

        def emit_exp(o, dsug, r):
            ds, ug = dsug
            src = flat3(ds) if ds is not None else flat3(ug[:, 0:T, :])
            if r == 0:
                nc.scalar.activation(flat(e[:, o]), src, ACTF.Exp)
            else:
                et = etp.tile([128, T, B], BF16, name="et", tag="et")
                nc.scalar.activation(flat(et), src, ACTF.Exp)
                eeng = nc.gpsimd if E_ENG[o] == "g" else nc.vector
                eeng.tensor_tensor(flat(e[:, o]), flat(e[:, o]),
                                   flat(et), op=ALU.mult)

        def emit_zpair(q):
            """partial softmax sums on Pool, overlapped with m2."""
            zq = zp.tile([128, T, B], BF16, name="zq", tag=f"z{q}", bufs=1)
            nc.gpsimd.tensor_tensor(flat(zq), flat(e[:, 2 * q]),
                                    flat(e[:, 2 * q + 1]), op=ALU.add)
            _zpart.append(zq)
            if q in (1, 3):  # fold pairs into quads as soon as available
                zz = zp.tile([128, T, B], BF16, name="zz", tag=f"zz{q}",
                             bufs=1)
                nc.gpsimd.tensor_tensor(flat(zz), flat(_zpart[-2]),
                                        flat(_zpart[-1]), op=ALU.add)
                _zquad.append(zz)

        DS_SHIFT = int(os.environ.get("DS_SHIFT", "3"))

        def m2(r, it):
            """delta_o for all o -> e (pass r), software-pipelined.
            Caller has emitted squash(it,0)+transposes(0); squash/transposes
            of the second half are interleaved after G(1)."""
            ugs = {}
            dss = {}
            for step in range(O + DS_SHIFT + 1):
                if step < O:
                    ugs[step] = emit_G(step, ROUTES[step])
                if step == 1:
                    squash(it, 1)
                    transposes(1)
                if 0 <= step - DS_SHIFT < O:
                    dss[step - DS_SHIFT] = emit_ds(
                        step - DS_SHIFT, ugs.pop(step - DS_SHIFT))
                if 0 <= step - DS_SHIFT - 1 < O:
                    oo = step - DS_SHIFT - 1
                    emit_exp(oo, dss.pop(oo), r)
                    if oo % 2 == 1:
                        emit_zpair(oo // 2)
                if step == 3 and len(_zpart) >= 2:
                    pass

        def softmax_tail():
            """finish Z = sum_o e; rz = 1/Z (bf16)."""
            za = zp.tile([128, T, B], BF16, name="za", tag="za")
            nc.vector.tensor_tensor(flat(za), flat(_zquad[0]),
                                    flat(_zquad[1]), op=ALU.add)
            nc.vector.tensor_tensor(flat(za), flat(za), flat(_zpart[4]),
                                    op=ALU.add)
            with nc.allow_low_precision("softmax normalizer, 2e-2 tolerance"):
                nc.vector.reciprocal(flat(rzb), flat(za))
            _zpart.clear()
            _zquad.clear()

        def m1_B(it):
            """s[b, o, j] = sum_{i,d} (c_o u) W for all o.
            squash/transposes of half 0 are emitted after o=4's chain."""
            for o in range(O):
                co = cop.tile([128, T, B], BF16, name="co", tag="co")
                ceng = nc.gpsimd if C_ENG[(it - 1) * O + o] == "g" else nc.vector
                ceng.tensor_tensor(flat(co), flat(e[:, o]), flat(rzb),
                                   op=ALU.mult)
                cu = cup.tile([128, T, D, B], BF16, name="cu", tag="cu")
                cueng = nc.gpsimd if CU_ENG[(it - 1) * O + o] == "g" else nc.vector
                cueng.tensor_tensor(
                    cu[:, :, :, :],
                    co.unsqueeze(2).broadcast_to([128, T, D, B]),
                    ui[:, :, :, :], op=ALU.mult)
                for t in range(T):
                    for d in range(D):
                        td = t * D + d
                        nc.tensor.matmul(
                            s_ps[:, o, :], cu[:, t, d, :], ws[:, t, d, o, :],
                            start=(td == 0), stop=(td == KC - 1),
                            tile_position=(0, 0), skip_group_check=True,
                        )
                if o == 4:
                    squash(it, 0)
                    if it < 2:
                        transposes(0)
                    else:
                        nc.sync.dma_start(out=vout_d[:, 0:5, :],
                                          in_=v_f[:, 0:5, :])

        _zpart = []
        _zquad = []

        # ========================= flow =========================
        W1 = int(os.environ.get("WARM1", "25"))
        W2 = int(os.environ.get("WARM2", "10"))
        m1_A_chain(0)
        squash(0, 0)
        m1_A_chain(1)
        transposes(0)
        for r in range(2):
            m2(r, r)
            pe_keepalive(W1)
            softmax_tail()
            m1_B(r + 1)
            pe_keepalive(W2)
        squash(2, 1)
        nc.sync.dma_start(out=vout_d[:, 5:10, :], in_=v_f[:, 5:10, :])

    nc.finalize()
    return nc


def _host_prep(u, weights):
    """Per-core input maps. u [512,1152,8] f32, weights [1152,10,16,8] f32."""
    W = np.asarray(weights, dtype=np.float32)
    u = np.asarray(u, dtype=np.float32)
    # ws[p, t, d, o, j] = W[t*128+p, o, j, d]
    ws = np.ascontiguousarray(
        W.reshape(T, 128, O, J, D).transpose(1, 0, 4, 2, 3)
    ).astype(bfnp)
    # wb2[32g+jj, sl, kc, m] = W[c*128+m, o, jj, d], kc = d*T + c
    wt = W.reshape(T, 128, O, J, D)  # [c, m, o, j, d]
    wb2 = np.zeros((128, 3, KC, 128), dtype=bfnp)
    for o in range(O):
        g, sl = o % 4, o // 4
        blk = wt[:, :, o, :, :].transpose(2, 3, 0, 1)  # [j, d, c, m]
        wb2[32 * g : 32 * g + 16, sl] = blk.reshape(J, KC, 128).astype(bfnp)
    id128 = np.eye(128, dtype=np.float32).astype(bfnp)
    id64 = np.eye(64, dtype=np.float32).astype(bfnp)

    base = {"ws": ws, "wb2": wb2, "id128": id128, "id64": id64}
    in_maps = []
    for c in range(N_CORES):
        uc = u[c * B : (c + 1) * B]  # [64, 1152, 8]
        ur = uc.reshape(B, T, 128, D)
        ui = np.ascontiguousarray(ur.transpose(2, 1, 3, 0)).astype(bfnp)
        # ui2[p, kc, b] = u[b, c*128+p, d], kc = d*T + c
        ui2 = np.ascontiguousarray(
            ur.transpose(2, 3, 1, 0).reshape(128, D * T, B)
        ).astype(bfnp)
        in_maps.append({**base, "ui": ui, "ui2": ui2})
    return in_maps


def kernel(u, weights):
    if "nc" not in _cache:
        _cache["nc"] = build_nc()
    nc = _cache["nc"]
    in_maps = _host_prep(u, weights)
    res = run_bass_kernel_spmd(nc, in_maps, core_ids=list(range(N_CORES)))
    out = np.concatenate([res.results[c]["vout"] for c in range(N_CORES)], axis=0)
    return out.astype(np.float32)


if __name__ == "__main__":
    rng = np.random.default_rng(0)
    u = rng.standard_normal((512, 1152, 8), dtype=np.float32)
    w = (rng.standard_normal((1152, 10, 16, 8)) * 0.1).astype(np.float32)
    v = kernel(u, w)
    print("out", v.shape, v.dtype, np.abs(v).max())


# revision 10
# speedup vs baseline: 1.0203x; 1.0181x over previous
"""CapsNet dynamic-routing layer on 8 Trainium2 NeuronCores (Bass/Tile), v2.

reference math (per batch element b):
  u_hat[b,i,o,j] = sum_d W[i,o,j,d] * u[b,i,d]        (never materialized)
  bl = 0; for r in 0..2:
    c = softmax_o(bl); s[b,o,j] = sum_i c*u_hat; v = squash(s)
    if r < 2: bl += sum_j u_hat*v
  return v  [B, 10, 16]

Distribution: pure data parallel, batch 512 -> 64 per core x 8 cores;
weights replicated.  Per-core: b=64, i=1152=9*128, o=10, j=16, d=8.

v2 design vs v1:
  * m1 (s-matmuls) output b-partitioned [64, (o,j)] -> n=16/o per
    instruction instead of n=64: 4x fewer PE rows.
  * m2 (agreement) G^T = W^T v computed (d,i)-partitioned: out
    [128=(d,i)chunk, 64 b] per kc-chunk: 2x fewer PE rows than v1.
  * d-reduction done ON PE via chained identity matmuls accumulating
    in PSUM (start/stop over d) - removes the DVE add tree entirely.
  * logits bl never materialized: e2 = e1 * exp(delta2) folds the
    cross-iteration accumulation into the exp chain.
  * softmax stays i-partitioned end-to-end: no DMA transposes.
  * squash in b-partitioned layout: a handful of [64, 10]-sized ops.
  * PSUM->SBUF conversion work (f32 G -> bf16 for 2x-mode DVE mult)
    is routed per-o across Act / DVE-direct / Pool to balance engines.
"""
import sys

sys.path.insert(0, "/opt/trn_rl_repo")

import numpy as np
import ml_dtypes
from contextlib import ExitStack

from concourse import bacc, mybir, hw_specs
from concourse.tile import TileContext
from concourse.bass_utils import run_bass_kernel_spmd

BF16 = mybir.dt.bfloat16
F32 = mybir.dt.float32
AX = mybir.AxisListType
ALU = mybir.AluOpType
ACTF = mybir.ActivationFunctionType
bfnp = ml_dtypes.bfloat16

B = 64
I = 1152
T = 9          # i-chunks of 128 (also the softmax "c" index)
O = 10
J = 16
D = 8
KC = T * D     # 72 k-chunks of 128 over flat k = d*I + i
EPS = 1e-06
N_CORES = 8

_cache = {}

# Route every activation through the one table set that has exp+ln+copy,
# so the ACT engine never reloads tables mid-kernel.
_KEEP_SET = "natural_log_exp_and_others"


def _patched_tables(arch):
    full = {k: set(v) for k, v in hw_specs.get_activation_tables(arch).items()}
    keep = full[_KEEP_SET]
    return {k: (v if k == _KEEP_SET else v - keep) for k, v in full.items()}


import os
if os.environ.get('ACT_PATCH', '1') == '1':
    bacc.get_activation_tables = _patched_tables

# per-o conversion route for the m2 ug-mult (GPSIMD cannot read PSUM,
# so conversions are Act or DVE only):
#   'a' = Act copies PSUM f32 -> SBUF bf16, DVE multiplies at 2x
#   'A' = Act copies, Pool multiplies (SBUF-only, legal)
#   'b' = DVE multiplies straight from PSUM f32 (1x, no Act work)
#   'm' = per-chunk alternation: even chunks Act-copy, odd chunks
#         DVE-direct; DVE strided 2x mult for the copied half
ROUTES = os.environ.get("M2_ROUTES", "MMMMMMMMMM")
# per-o d-sum engine: 'T' PE identity-matmul chain (psum f32)
#                     'v' DVE in-place bf16 fold tree | 'g' Pool fold tree
DS_ENG = os.environ.get("DS_ENG", "TgTTvTTgvT")
# per-(it,o) cu engine: 'v' DVE | 'g' Pool  (20 chars: it1 o0-9, it2 o0-9)
CU_ENG = os.environ.get("CU_ENG", "s" * 20)
# engine for per-o c = e*rz mult: 'v' DVE | 'g' Pool
C_ENG = os.environ.get("C_ENG", "gv" * 10)
if len(C_ENG) == 1:
    C_ENG = C_ENG * 20
# per-o count of Act-copied chunk-groups in m/M routes (rest DVE-direct)
M_H = os.environ.get("M_H", "5555545545")
# engine for v_jb strip copies: 'a' Act | 'v' DVE
VJB_ENG = os.environ.get("VJB_ENG", "v")
# per-o engine for the pass-2 e = e*exp(delta) mult: 'v' DVE | 'g' Pool
E_ENG = os.environ.get("E_ENG", "vgvgvgvgvg")
SPLIT_MULT = os.environ.get("SPLIT_MULT", "1") == "1"
DVE_FIRST = os.environ.get("DVE_FIRST", "0") == "1"
CU_CUT = int(os.environ.get("CU_CUT", "40"))


def build_nc():
    nc = bacc.Bacc()
    ws_d = nc.dram_tensor("ws", [128, T, D, O, J], BF16, kind="ExternalInput")
    ui_d = nc.dram_tensor("ui", [128, T, D, B], BF16, kind="ExternalInput")
    ui2_d = nc.dram_tensor("ui2", [128, KC, B], BF16, kind="ExternalInput")
    wb2_d = nc.dram_tensor("wb2", [128, 3, KC, 128], BF16, kind="ExternalInput")
    id128_d = nc.dram_tensor("id128", [128, 128], BF16, kind="ExternalInput")
    id64_d = nc.dram_tensor("id64", [64, 64], BF16, kind="ExternalInput")
    vout_d = nc.dram_tensor("vout", [B, O, J], F32, kind="ExternalOutput")

    with TileContext(nc) as tc, ExitStack() as ctx:
        static = ctx.enter_context(tc.tile_pool(name="static", bufs=1))
        work = ctx.enter_context(tc.tile_pool(name="work", bufs=1))
        gsbp = ctx.enter_context(tc.tile_pool(name="gsbp", bufs=2))
        ugp = ctx.enter_context(tc.tile_pool(
            name="ugp", bufs=int(os.environ.get("DS_SHIFT", "3")) + 2))
        cup = ctx.enter_context(tc.tile_pool(name="cup", bufs=2))
        etp = ctx.enter_context(tc.tile_pool(name="etp", bufs=2))
        cop = ctx.enter_context(tc.tile_pool(name="cop", bufs=2))
        zp = ctx.enter_context(tc.tile_pool(name="zp", bufs=2))
        psS = ctx.enter_context(tc.tile_pool(name="psS", bufs=1, space="PSUM"))
        psVT = ctx.enter_context(tc.tile_pool(name="psVT", bufs=1, space="PSUM"))
        psG = ctx.enter_context(tc.tile_pool(
            name="psG", bufs=int(os.environ.get("PSG_BUFS", "4")), space="PSUM"))
        psDS = ctx.enter_context(tc.tile_pool(
            name="psDS", bufs=int(os.environ.get("PSDS_BUFS", "1")), space="PSUM"))

        # PE p-state: the clock ramps per continuous-busy stretch (reset on
        # idle; full speed only after 3us busy).  Dummy matmuls keep the PE
        # clock hot through DMA waits and phase boundaries.
        warm = static.tile([128, 128], BF16, name="warm")
        nc.vector.memset(warm, 0.0)

        def pe_keepalive(n):
            for _ in range(n):
                wps = psVT.tile([128, 96], F32, name="wps", tag="vt0")
                nc.tensor.matmul(wps, warm, warm[:, 0:96], start=True,
                                 stop=True, tile_position=(0, 0))

        pe_keepalive(int(os.environ.get("WARM0", "75")))

        ws = static.tile([128, T, D, O, J], BF16, name="ws")
        ui = static.tile([128, T, D, B], BF16, name="ui")
        ui2 = static.tile([128, KC, B], BF16, name="ui2")
        wb2 = static.tile([128, 3, KC, 128], BF16, name="wb2")
        id128 = static.tile([128, 128], BF16, name="id128")
        id64 = static.tile([64, 64], BF16, name="id64")
        eps1 = static.tile([64, 1], F32, name="eps1")
        nc.vector.memset(eps1, EPS)

        # DMA cost model (legacy CoreSim): each DMA holds the issuing
        # engine's queue for ~1.7us fixed + per-partition-free-bytes *
        # 0.39ns.  So: few big DMAs, spread across the SP / Act / Pool
        # queues, ordered by first use.
        # SP:   ws t0-4, id64, wb2 slot1, wb2 slot2
        # Pool: ws t5-8, wb2 slot0, id128
        # Act:  ui, ui2   (Act must be free for squash-0 at ~12us)
        if os.environ.get("DMA_PLAN", "A") == "A":
            nc.sync.dma_start(out=ws[:, 0:5], in_=ws_d[:, 0:5])
            nc.gpsimd.dma_start(out=ws[:, 5:9], in_=ws_d[:, 5:9])
            nc.scalar.dma_start(out=ui, in_=ui_d[:, :])
            nc.sync.dma_start(out=id64, in_=id64_d[:, :])
            nc.gpsimd.dma_start(out=wb2[:, 0], in_=wb2_d[:, 0])
            nc.gpsimd.dma_start(out=id128, in_=id128_d[:, :])
            nc.scalar.dma_start(out=ui2, in_=ui2_d[:, :])
            nc.sync.dma_start(out=wb2[:, 1], in_=wb2_d[:, 1])
            nc.sync.dma_start(out=wb2[:, 2], in_=wb2_d[:, 2])
        else:
            # plan E: per-t ws chunks alternating SP/Pool so the m1_A chain
            # streams at DMA pitch; slot0 split across both queues after.
            for t in range(0, 9, 2):
                nc.sync.dma_start(out=ws[:, t], in_=ws_d[:, t])
            for t in range(1, 9, 2):
                nc.gpsimd.dma_start(out=ws[:, t], in_=ws_d[:, t])
            nc.scalar.dma_start(out=ui, in_=ui_d[:, :])
            nc.scalar.dma_start(out=id64, in_=id64_d[:, :])
            nc.sync.dma_start(out=wb2[:, 0, 0:36], in_=wb2_d[:, 0, 0:36])
            nc.gpsimd.dma_start(out=wb2[:, 0, 36:72], in_=wb2_d[:, 0, 36:72])
            nc.scalar.dma_start(out=ui2, in_=ui2_d[:, :])
            nc.sync.dma_start(out=id128, in_=id128_d[:, :])
            nc.sync.dma_start(out=wb2[:, 1], in_=wb2_d[:, 1])
            nc.gpsimd.dma_start(out=wb2[:, 2], in_=wb2_d[:, 2])

        # persistent work tiles
        e = work.tile([128, O, T, B], BF16, name="e")
        rz_f = work.tile([128, T, B], F32, name="rz_f")
        rzb = work.tile([128, T, B], BF16, name="rzb")
        v_f = work.tile([64, O, J], F32, name="v_f")
        v_b = work.tile([64, O, J], BF16, name="v_b")
        v_jb = work.tile([128, 3, B], BF16, name="v_jb")
        s2 = work.tile([64, O, J], F32, name="s2")
        sq = work.tile([64, O], F32, name="sq")
        t1 = work.tile([64, O], F32, name="t1")
        den = work.tile([64, O], F32, name="den")
        rcp = work.tile([64, O], F32, name="rcp")
        ff = work.tile([64, O], F32, name="ff")

        s_ps = psS.tile([64, O, J], F32, name="s_ps")

        def m1_A_chain(h):
            """it0 half h: c uniform -> s_raw[b, o-half] = sum_{i,d} W u."""
            o5 = slice(5 * h, 5 * h + 5)
            for t in range(T):
                for d in range(D):
                    td = t * D + d
                    nc.tensor.matmul(
                        s_ps[:, o5, :].rearrange("p o j -> p (o j)"),
                        ui[:, t, d, :],
                        ws[:, t, d, o5, :].rearrange("p o j -> p (o j)"),
                        start=(td == 0), stop=(td == KC - 1),
                        tile_position=(0, 0), skip_group_check=True,
                    )

        def squash(it, h):
            """v[:, half] = squash(scale * s_ps[:, half]), tiny b-part ops."""
            scale = 0.1 if it == 0 else 1.0
            o5 = slice(5 * h, 5 * h + 5)
            nc.scalar.activation(s2[:, o5, :], s_ps[:, o5, :], ACTF.Square,
                                 scale=scale)
            nc.vector.tensor_reduce(sq[:, o5], s2[:, o5, :], axis=AX.X,
                                    op=ALU.add)
            nc.scalar.activation(t1[:, o5], sq[:, o5], ACTF.Ln, bias=eps1)
            nc.scalar.activation(den[:, o5], t1[:, o5], ACTF.Exp, scale=0.5)
            nc.vector.tensor_scalar_add(t1[:, o5], sq[:, o5], 1.0)
            nc.vector.tensor_tensor(den[:, o5], den[:, o5], t1[:, o5],
                                    op=ALU.mult)
            nc.vector.reciprocal(rcp[:, o5], den[:, o5])
            nc.vector.tensor_tensor(ff[:, o5], sq[:, o5], rcp[:, o5],
                                    op=ALU.mult)
            if it == 0:
                nc.vector.tensor_scalar_mul(ff[:, o5], ff[:, o5], scale)
            nc.vector.tensor_tensor(
                v_f[:, o5, :], s_ps[:, o5, :],
                ff[:, o5].unsqueeze(2).broadcast_to([64, 5, J]),
                op=ALU.mult)
            nc.vector.tensor_copy(
                v_b[:, o5, :].rearrange("p o j -> p (o j)"),
                v_f[:, o5, :].rearrange("p o j -> p (o j)"))

        # half-h (g, sl) slots are disjoint: h0 -> sl0 strips + (g0, sl1);
        # h1 -> (g1..3, sl1) + (g0..1, sl2).
        def transposes(h):
            vt = psVT.tile([128, 3, B], BF16, name="vt", tag="vt0",
                           bufs=1)
            for o in range(5 * h, 5 * h + 5):
                g, sl = o % 4, o // 4
                nc.tensor.matmul(
                    vt[32 * g : 32 * g + 16, sl, :],
                    v_b[:, o, :], id64,
                    is_transpose=True, tile_position=(0, 32 * g),
                )
            for o in range(5 * h, 5 * h + 5):
                g, sl = o % 4, o // 4
                if VJB_ENG == "a":
                    nc.scalar.copy(v_jb[32 * g : 32 * g + 16, sl, :],
                                   vt[32 * g : 32 * g + 16, sl, :])
                else:
                    nc.vector.tensor_copy(
                        v_jb[32 * g : 32 * g + 16, sl, :],
                        vt[32 * g : 32 * g + 16, sl, :])

        flat = lambda ap: ap.rearrange("p t b -> p (t b)")
        flat3 = lambda ap: ap.rearrange("p a b -> p (a b)")

        def emit_G(o, route):
            """G^T chunks for o; returns the ug tile being filled."""
            g, sl = o % 4, o // 4
            ug = ugp.tile([128, KC, B], BF16, name="ug", tag="ug")
            gsb = None
            if route != "b":
                gsb = gsbp.tile([128, KC, B], BF16, name="gsb", tag="gsb")

            def gmm(pg, kk, kc):
                nc.tensor.matmul(
                    pg[:, kk, :],
                    wb2[32 * g : 32 * g + 16, sl, kc, :],
                    v_jb[32 * g : 32 * g + 16, sl, :],
                    start=True, stop=True,
                    tile_position=(32 * g, 0),
                )

            if route == "6":
                # 16-kc psG tiles: (Act, Act, DVE, DVE, Act-half); Pool
                # multiplies the Act-copied parts.
                for ti in range(5):
                    k0 = 16 * ti
                    nk = 16 if ti < 4 else 8
                    pg = psG.tile([128, 16, B], F32, name="pg", tag="pg")
                    for kk in range(nk):
                        gmm(pg, kk, k0 + kk)
                    slk = slice(k0, k0 + nk)
                    if ti in (0, 1, 4):
                        nc.scalar.copy(flat3(gsb[:, slk, :]),
                                       flat3(pg[:, 0:nk, :]))
                    else:
                        nc.vector.tensor_tensor(
                            flat3(ug[:, slk, :]), flat3(pg[:, 0:nk, :]),
                            flat3(ui2[:, slk, :]), op=ALU.mult)
                nc.gpsimd.tensor_tensor(
                    flat3(ug[:, 0:32, :]), flat3(gsb[:, 0:32, :]),
                    flat3(ui2[:, 0:32, :]), op=ALU.mult)
                nc.gpsimd.tensor_tensor(
                    flat3(ug[:, 64:72, :]), flat3(gsb[:, 64:72, :]),
                    flat3(ui2[:, 64:72, :]), op=ALU.mult)
                return ug

            nA = int(M_H[o]) if route in ("m", "M") else 9
            horder = list(range(9))
            if DVE_FIRST and route in ("m", "M"):
                horder = list(range(nA, 9)) + list(range(nA))
            for h in horder:
                pg = psG.tile([128, 8, B], F32, name="pg", tag="pg")
                for kk in range(8):
                    gmm(pg, kk, 8 * h + kk)
                sl8 = slice(8 * h, 8 * h + 8)
                if route in ("a", "A") or (route in ("m", "M") and h < nA):
                    nc.scalar.copy(flat3(gsb[:, sl8, :]),
                                   flat3(pg[:, 0:8, :]))
                else:  # DVE straight from PSUM
                    nc.vector.tensor_tensor(
                        flat3(ug[:, sl8, :]), flat3(pg[:, 0:8, :]),
                        flat3(ui2[:, sl8, :]), op=ALU.mult)
            if route in ("a", "A"):
                meng = nc.gpsimd if route == "A" else nc.vector
                meng.tensor_tensor(flat3(ug), flat3(gsb), flat3(ui2),
                                   op=ALU.mult)
            elif route in ("m", "M"):
                meng = nc.gpsimd if route == "M" else nc.vector
                if SPLIT_MULT and nA >= 3:
                    # two halves so the first can run while the later Act
                    # copies are still in flight
                    cut = 8 * (nA // 2 + 1)
                    for sl_ in (slice(0, cut), slice(cut, 8 * nA)):
                        meng.tensor_tensor(
                            flat3(ug[:, sl_, :]), flat3(gsb[:, sl_, :]),
                            flat3(ui2[:, sl_, :]), op=ALU.mult)
                else:
                    hA = slice(0, 8 * nA)
                    meng.tensor_tensor(
                        flat3(ug[:, hA, :]), flat3(gsb[:, hA, :]),
                        flat3(ui2[:, hA, :]), op=ALU.mult)
            return ug

        def emit_ds(o, ug):
            """delta[o] = sum_d ug chunks.  Returns (psum_tile|None, ug)."""
            eng = DS_ENG[o]
            if eng == "T":  # PE identity-matmul chains into PSUM f32
                ds = psDS.tile([128, T, B], F32, name="ds", tag="ds")
                for d in range(D):
                    nc.tensor.matmul(
                        flat3(ds[:, 0:8, :]), id128,
                        flat3(ug[:, d * T : d * T + 8, :]),
                        start=(d == 0), stop=(d == D - 1),
                        tile_position=(0, 0), skip_group_check=True,
                    )
                for d in range(D):
                    nc.tensor.matmul(
                        ds[:, 8, :], id128, ug[:, d * T + 8, :],
                        start=(d == 0), stop=(d == D - 1),
                        tile_position=(0, 0), skip_group_check=True,
                    )
                return ds, ug
            if eng == "x":
                # each fold split across DVE and Pool (disjoint halves)
                for w in (36, 18, 9):
                    cut = (w * 5) // 8
                    nc.vector.tensor_tensor(
                        flat3(ug[:, 0:cut, :]), flat3(ug[:, 0:cut, :]),
                        flat3(ug[:, w : w + cut, :]), op=ALU.add)
                    nc.gpsimd.tensor_tensor(
                        flat3(ug[:, cut:w, :]), flat3(ug[:, cut:w, :]),
                        flat3(ug[:, w + cut : 2 * w, :]), op=ALU.add)
                return None, ug
            ve = nc.vector if eng == "v" else nc.gpsimd
            # in-place bf16 fold tree: 72 -> 36 -> 18 -> 9 chunks
            for w in (36, 18, 9):
                ve.tensor_tensor(flat3(ug[:, 0:w, :]), flat3(ug[:, 0:w, :]),
                                 flat3(ug[:, w : 2 * w, :]), op=ALU.add)
            return None, ug

        def emit_exp(o, dsug, r):
            ds, ug = dsug
            src = flat3(ds) if ds is not None else flat3(ug[:, 0:T, :])
            if r == 0:
                nc.scalar.activation(flat(e[:, o]), src, ACTF.Exp)
            else:
                et = etp.tile([128, T, B], BF16, name="et", tag="et")
                nc.scalar.activation(flat(et), src, ACTF.Exp)
                eeng = nc.gpsimd if E_ENG[o] == "g" else nc.vector
                eeng.tensor_tensor(flat(e[:, o]), flat(e[:, o]),
                                   flat(et), op=ALU.mult)

        def emit_zpair(q):
            """partial softmax sums on Pool, overlapped with m2."""
            zq = zp.tile([128, T, B], BF16, name="zq", tag=f"z{q}", bufs=1)
            nc.gpsimd.tensor_tensor(flat(zq), flat(e[:, 2 * q]),
                                    flat(e[:, 2 * q + 1]), op=ALU.add)
            _zpart.append(zq)
            if q in (1, 3):  # fold pairs into quads as soon as available
                zz = zp.tile([128, T, B], BF16, name="zz", tag=f"zz{q}",
                             bufs=1)
                nc.gpsimd.tensor_tensor(flat(zz), flat(_zpart[-2]),
                                        flat(_zpart[-1]), op=ALU.add)
                _zquad.append(zz)

        DS_SHIFT = int(os.environ.get("DS_SHIFT", "3"))

        def m2(r, it):
            """delta_o for all o -> e (pass r), software-pipelined.
            Caller has emitted squash(it,0)+transposes(0); squash/transposes
            of the second half are interleaved after G(1)."""
            ugs = {}
            dss = {}
            for step in range(O + DS_SHIFT + 1):
                if step < O:
                    ugs[step] = emit_G(step, ROUTES[step])
                if step == 1:
                    squash(it, 1)
                    transposes(1)
                if 0 <= step - DS_SHIFT < O:
                    dss[step - DS_SHIFT] = emit_ds(
                        step - DS_SHIFT, ugs.pop(step - DS_SHIFT))
                if 0 <= step - DS_SHIFT - 1 < O:
                    oo = step - DS_SHIFT - 1
                    emit_exp(oo, dss.pop(oo), r)
                    if oo % 2 == 1:
                        emit_zpair(oo // 2)
                if step == 3 and len(_zpart) >= 2:
                    pass

        def softmax_tail():
            """finish Z = sum_o e; rz = 1/Z (bf16)."""
            za = zp.tile([128, T, B], BF16, name="za", tag="za")
            nc.vector.tensor_tensor(flat(za), flat(_zquad[0]),
                                    flat(_zquad[1]), op=ALU.add)
            nc.vector.tensor_tensor(flat(za), flat(za), flat(_zpart[4]),
                                    op=ALU.add)
            with nc.allow_low_precision("softmax normalizer, 2e-2 tolerance"):
                nc.vector.reciprocal(flat(rzb), flat(za))
            _zpart.clear()
            _zquad.clear()

        def m1_B(it):
            """s[b, o, j] = sum_{i,d} (c_o u) W for all o.
            squash/transposes of half 0 are emitted after o=4's chain."""
            for o in range(O):
                co = cop.tile([128, T, B], BF16, name="co", tag="co")
                ceng = nc.gpsimd if C_ENG[(it - 1) * O + o] == "g" else nc.vector
                ceng.tensor_tensor(flat(co), flat(e[:, o]), flat(rzb),
                                   op=ALU.mult)
                cu = cup.tile([128, T, D, B], BF16, name="cu", tag="cu")
                ch = CU_ENG[(it - 1) * O + o]
                if ch == "s":
                    # split the multiply across DVE and Pool (rate-matched)
                    cut = CU_CUT
                    nc.vector.tensor_tensor(
                        cu[:, :, :, 0:cut],
                        co[:, :, 0:cut].unsqueeze(2).broadcast_to(
                            [128, T, D, cut]),
                        ui[:, :, :, 0:cut], op=ALU.mult)
                    nc.gpsimd.tensor_tensor(
                        cu[:, :, :, cut:B],
                        co[:, :, cut:B].unsqueeze(2).broadcast_to(
                            [128, T, D, B - cut]),
                        ui[:, :, :, cut:B], op=ALU.mult)
                else:
                    cueng = nc.gpsimd if ch == "g" else nc.vector
                    cueng.tensor_tensor(
                        cu[:, :, :, :],
                        co.unsqueeze(2).broadcast_to([128, T, D, B]),
                        ui[:, :, :, :], op=ALU.mult)
                for t in range(T):
                    for d in range(D):
                        td = t * D + d
                        nc.tensor.matmul(
                            s_ps[:, o, :], cu[:, t, d, :], ws[:, t, d, o, :],
                            start=(td == 0), stop=(td == KC - 1),
                            tile_position=(0, 0), skip_group_check=True,
                        )
                if o == 4:
                    squash(it, 0)
                    if it < 2:
                        transposes(0)
                    else:
                        nc.sync.dma_start(out=vout_d[:, 0:5, :],
                                          in_=v_f[:, 0:5, :])

        _zpart = []
        _zquad = []

        # ========================= flow =========================
        W1 = int(os.environ.get("WARM1", "25"))
        W2 = int(os.environ.get("WARM2", "10"))
        m1_A_chain(0)
        squash(0, 0)
        m1_A_chain(1)
        transposes(0)
        for r in range(2):
            m2(r, r)
            pe_keepalive(W1)
            softmax_tail()
            m1_B(r + 1)
            pe_keepalive(W2)
        squash(2, 1)
        nc.sync.dma_start(out=vout_d[:, 5:10, :], in_=v_f[:, 5:10, :])

    nc.finalize()
    return nc


def _host_prep(u, weights):
    """Per-core input maps. u [512,1152,8] f32, weights [1152,10,16,8] f32."""
    W = np.asarray(weights, dtype=np.float32)
    u = np.asarray(u, dtype=np.float32)
    # ws[p, t, d, o, j] = W[t*128+p, o, j, d]
    ws = np.ascontiguousarray(
        W.reshape(T, 128, O, J, D).transpose(1, 0, 4, 2, 3)
    ).astype(bfnp)
    # wb2[32g+jj, sl, kc, m] = W[c*128+m, o, jj, d], kc = d*T + c
    wt = W.reshape(T, 128, O, J, D)  # [c, m, o, j, d]
    wb2 = np.zeros((128, 3, KC, 128), dtype=bfnp)
    for o in range(O):
        g, sl = o % 4, o // 4
        blk = wt[:, :, o, :, :].transpose(2, 3, 0, 1)  # [j, d, c, m]
        wb2[32 * g : 32 * g + 16, sl] = blk.reshape(J, KC, 128).astype(bfnp)
    id128 = np.eye(128, dtype=np.float32).astype(bfnp)
    id64 = np.eye(64, dtype=np.float32).astype(bfnp)

    base = {"ws": ws, "wb2": wb2, "id128": id128, "id64": id64}
    in_maps = []
    for c in range(N_CORES):
        uc = u[c * B : (c + 1) * B]  # [64, 1152, 8]
        ur = uc.reshape(B, T, 128, D)
        ui = np.ascontiguousarray(ur.transpose(2, 1, 3, 0)).astype(bfnp)
        # ui2[p, kc, b] = u[b, c*128+p, d], kc = d*T + c
        ui2 = np.ascontiguousarray(
            ur.transpose(2, 3, 1, 0).reshape(128, D * T, B)
        ).astype(bfnp)
        in_maps.append({**base, "ui": ui, "ui2": ui2})
    return in_maps


def kernel(u, weights):
    if "nc" not in _cache:
        _cache["nc"] = build_nc()
    nc = _cache["nc"]
    in_maps = _host_prep(u, weights)
    res = run_bass_kernel_spmd(nc, in_maps, core_ids=list(range(N_CORES)))
    out = np.concatenate([res.results[c]["vout"] for c in range(N_CORES)], axis=0)
    return out.astype(np.float32)


if __name__ == "__main__":
    rng = np.random.default_rng(0)
    u = rng.standard_normal((512, 1152, 8), dtype=np.float32)
    w = (rng.standard_normal((1152, 10, 16, 8)) * 0.1).astype(np.float32)
    v = kernel(u, w)
    print("out", v.shape, v.dtype, np.abs(v).max())


# revision 11
# speedup vs baseline: 1.0223x; 1.0019x over previous
"""CapsNet dynamic-routing layer on 8 Trainium2 NeuronCores (Bass/Tile), v2.

reference math (per batch element b):
  u_hat[b,i,o,j] = sum_d W[i,o,j,d] * u[b,i,d]        (never materialized)
  bl = 0; for r in 0..2:
    c = softmax_o(bl); s[b,o,j] = sum_i c*u_hat; v = squash(s)
    if r < 2: bl += sum_j u_hat*v
  return v  [B, 10, 16]

Distribution: pure data parallel, batch 512 -> 64 per core x 8 cores;
weights replicated.  Per-core: b=64, i=1152=9*128, o=10, j=16, d=8.

v2 design vs v1:
  * m1 (s-matmuls) output b-partitioned [64, (o,j)] -> n=16/o per
    instruction instead of n=64: 4x fewer PE rows.
  * m2 (agreement) G^T = W^T v computed (d,i)-partitioned: out
    [128=(d,i)chunk, 64 b] per kc-chunk: 2x fewer PE rows than v1.
  * d-reduction done ON PE via chained identity matmuls accumulating
    in PSUM (start/stop over d) - removes the DVE add tree entirely.
  * logits bl never materialized: e2 = e1 * exp(delta2) folds the
    cross-iteration accumulation into the exp chain.
  * softmax stays i-partitioned end-to-end: no DMA transposes.
  * squash in b-partitioned layout: a handful of [64, 10]-sized ops.
  * PSUM->SBUF conversion work (f32 G -> bf16 for 2x-mode DVE mult)
    is routed per-o across Act / DVE-direct / Pool to balance engines.
"""
import sys

sys.path.insert(0, "/opt/trn_rl_repo")

import numpy as np
import ml_dtypes
from contextlib import ExitStack

from concourse import bacc, mybir, hw_specs
from concourse.tile import TileContext
from concourse.bass_utils import run_bass_kernel_spmd

BF16 = mybir.dt.bfloat16
F32 = mybir.dt.float32
AX = mybir.AxisListType
ALU = mybir.AluOpType
ACTF = mybir.ActivationFunctionType
bfnp = ml_dtypes.bfloat16

B = 64
I = 1152
T = 9          # i-chunks of 128 (also the softmax "c" index)
O = 10
J = 16
D = 8
KC = T * D     # 72 k-chunks of 128 over flat k = d*I + i
EPS = 1e-06
N_CORES = 8

_cache = {}

# Route every activation through the one table set that has exp+ln+copy,
# so the ACT engine never reloads tables mid-kernel.
_KEEP_SET = "natural_log_exp_and_others"


def _patched_tables(arch):
    full = {k: set(v) for k, v in hw_specs.get_activation_tables(arch).items()}
    keep = full[_KEEP_SET]
    return {k: (v if k == _KEEP_SET else v - keep) for k, v in full.items()}


import os
if os.environ.get('ACT_PATCH', '1') == '1':
    bacc.get_activation_tables = _patched_tables

# per-o conversion route for the m2 ug-mult (GPSIMD cannot read PSUM,
# so conversions are Act or DVE only):
#   'a' = Act copies PSUM f32 -> SBUF bf16, DVE multiplies at 2x
#   'A' = Act copies, Pool multiplies (SBUF-only, legal)
#   'b' = DVE multiplies straight from PSUM f32 (1x, no Act work)
#   'm' = per-chunk alternation: even chunks Act-copy, odd chunks
#         DVE-direct; DVE strided 2x mult for the copied half
ROUTES = os.environ.get("M2_ROUTES", "MMMMMMMMMM")
# per-o d-sum engine: 'T' PE identity-matmul chain (psum f32)
#                     'v' DVE in-place bf16 fold tree | 'g' Pool fold tree
DS_ENG = os.environ.get("DS_ENG", "TgTTvTTgvT")
# per-(it,o) cu engine: 'v' DVE | 'g' Pool  (20 chars: it1 o0-9, it2 o0-9)
CU_ENG = os.environ.get("CU_ENG", "s" * 20)
# engine for per-o c = e*rz mult: 'v' DVE | 'g' Pool
C_ENG = os.environ.get("C_ENG", "gv" * 10)
if len(C_ENG) == 1:
    C_ENG = C_ENG * 20
# per-o count of Act-copied chunk-groups in m/M routes (rest DVE-direct)
M_H = os.environ.get("M_H", "5555545545")
# engine for v_jb strip copies: 'a' Act | 'v' DVE
VJB_ENG = os.environ.get("VJB_ENG", "v")
# per-o engine for the pass-2 e = e*exp(delta) mult: 'v' DVE | 'g' Pool
E_ENG = os.environ.get("E_ENG", "vgvgvgvgvg")
SPLIT_MULT = os.environ.get("SPLIT_MULT", "1") == "1"
DVE_FIRST = os.environ.get("DVE_FIRST", "0") == "1"
CU_CUT = int(os.environ.get("CU_CUT", "39"))


def build_nc():
    nc = bacc.Bacc()
    ws_d = nc.dram_tensor("ws", [128, T, D, O, J], BF16, kind="ExternalInput")
    ui_d = nc.dram_tensor("ui", [128, T, D, B], BF16, kind="ExternalInput")
    ui2_d = nc.dram_tensor("ui2", [128, KC, B], BF16, kind="ExternalInput")
    wb2_d = nc.dram_tensor("wb2", [128, 3, KC, 128], BF16, kind="ExternalInput")
    id128_d = nc.dram_tensor("id128", [128, 128], BF16, kind="ExternalInput")
    id64_d = nc.dram_tensor("id64", [64, 64], BF16, kind="ExternalInput")
    vout_d = nc.dram_tensor("vout", [B, O, J], F32, kind="ExternalOutput")

    with TileContext(nc) as tc, ExitStack() as ctx:
        static = ctx.enter_context(tc.tile_pool(name="static", bufs=1))
        work = ctx.enter_context(tc.tile_pool(name="work", bufs=1))
        gsbp = ctx.enter_context(tc.tile_pool(name="gsbp", bufs=2))
        ugp = ctx.enter_context(tc.tile_pool(
            name="ugp", bufs=int(os.environ.get("DS_SHIFT", "3")) + 2))
        cup = ctx.enter_context(tc.tile_pool(name="cup", bufs=2))
        etp = ctx.enter_context(tc.tile_pool(name="etp", bufs=2))
        cop = ctx.enter_context(tc.tile_pool(name="cop", bufs=2))
        zp = ctx.enter_context(tc.tile_pool(name="zp", bufs=2))
        psS = ctx.enter_context(tc.tile_pool(name="psS", bufs=1, space="PSUM"))
        psVT = ctx.enter_context(tc.tile_pool(name="psVT", bufs=1, space="PSUM"))
        psG = ctx.enter_context(tc.tile_pool(
            name="psG", bufs=int(os.environ.get("PSG_BUFS", "4")), space="PSUM"))
        psDS = ctx.enter_context(tc.tile_pool(
            name="psDS", bufs=int(os.environ.get("PSDS_BUFS", "1")), space="PSUM"))

        # PE p-state: the clock ramps per continuous-busy stretch (reset on
        # idle; full speed only after 3us busy).  Dummy matmuls keep the PE
        # clock hot through DMA waits and phase boundaries.
        warm = static.tile([128, 128], BF16, name="warm")
        nc.vector.memset(warm, 0.0)

        def pe_keepalive(n):
            for _ in range(n):
                wps = psVT.tile([128, 96], F32, name="wps", tag="vt0")
                nc.tensor.matmul(wps, warm, warm[:, 0:96], start=True,
                                 stop=True, tile_position=(0, 0))

        pe_keepalive(int(os.environ.get("WARM0", "75")))

        ws = static.tile([128, T, D, O, J], BF16, name="ws")
        ui = static.tile([128, T, D, B], BF16, name="ui")
        ui2 = static.tile([128, KC, B], BF16, name="ui2")
        wb2 = static.tile([128, 3, KC, 128], BF16, name="wb2")
        id128 = static.tile([128, 128], BF16, name="id128")
        id64 = static.tile([64, 64], BF16, name="id64")
        eps1 = static.tile([64, 1], F32, name="eps1")
        nc.vector.memset(eps1, EPS)

        # DMA cost model (legacy CoreSim): each DMA holds the issuing
        # engine's queue for ~1.7us fixed + per-partition-free-bytes *
        # 0.39ns.  So: few big DMAs, spread across the SP / Act / Pool
        # queues, ordered by first use.
        # SP:   ws t0-4, id64, wb2 slot1, wb2 slot2
        # Pool: ws t5-8, wb2 slot0, id128
        # Act:  ui, ui2   (Act must be free for squash-0 at ~12us)
        if os.environ.get("DMA_PLAN", "A") == "A":
            nc.sync.dma_start(out=ws[:, 0:5], in_=ws_d[:, 0:5])
            nc.gpsimd.dma_start(out=ws[:, 5:9], in_=ws_d[:, 5:9])
            nc.scalar.dma_start(out=ui, in_=ui_d[:, :])
            nc.sync.dma_start(out=id64, in_=id64_d[:, :])
            nc.gpsimd.dma_start(out=wb2[:, 0], in_=wb2_d[:, 0])
            nc.gpsimd.dma_start(out=id128, in_=id128_d[:, :])
            nc.scalar.dma_start(out=ui2, in_=ui2_d[:, :])
            nc.sync.dma_start(out=wb2[:, 1], in_=wb2_d[:, 1])
            nc.sync.dma_start(out=wb2[:, 2], in_=wb2_d[:, 2])
        else:
            # plan E: per-t ws chunks alternating SP/Pool so the m1_A chain
            # streams at DMA pitch; slot0 split across both queues after.
            for t in range(0, 9, 2):
                nc.sync.dma_start(out=ws[:, t], in_=ws_d[:, t])
            for t in range(1, 9, 2):
                nc.gpsimd.dma_start(out=ws[:, t], in_=ws_d[:, t])
            nc.scalar.dma_start(out=ui, in_=ui_d[:, :])
            nc.scalar.dma_start(out=id64, in_=id64_d[:, :])
            nc.sync.dma_start(out=wb2[:, 0, 0:36], in_=wb2_d[:, 0, 0:36])
            nc.gpsimd.dma_start(out=wb2[:, 0, 36:72], in_=wb2_d[:, 0, 36:72])
            nc.scalar.dma_start(out=ui2, in_=ui2_d[:, :])
            nc.sync.dma_start(out=id128, in_=id128_d[:, :])
            nc.sync.dma_start(out=wb2[:, 1], in_=wb2_d[:, 1])
            nc.gpsimd.dma_start(out=wb2[:, 2], in_=wb2_d[:, 2])

        # persistent work tiles
        e = work.tile([128, O, T, B], BF16, name="e")
        rz_f = work.tile([128, T, B], F32, name="rz_f")
        rzb = work.tile([128, T, B], BF16, name="rzb")
        v_f = work.tile([64, O, J], F32, name="v_f")
        v_b = work.tile([64, O, J], BF16, name="v_b")
        v_jb = work.tile([128, 3, B], BF16, name="v_jb")
        s2 = work.tile([64, O, J], F32, name="s2")
        sq = work.tile([64, O], F32, name="sq")
        t1 = work.tile([64, O], F32, name="t1")
        den = work.tile([64, O], F32, name="den")
        rcp = work.tile([64, O], F32, name="rcp")
        ff = work.tile([64, O], F32, name="ff")

        s_ps = psS.tile([64, O, J], F32, name="s_ps")

        def m1_A_chain(h):
            """it0 half h: c uniform -> s_raw[b, o-half] = sum_{i,d} W u."""
            o5 = slice(5 * h, 5 * h + 5)
            for t in range(T):
                for d in range(D):
                    td = t * D + d
                    nc.tensor.matmul(
                        s_ps[:, o5, :].rearrange("p o j -> p (o j)"),
                        ui[:, t, d, :],
                        ws[:, t, d, o5, :].rearrange("p o j -> p (o j)"),
                        start=(td == 0), stop=(td == KC - 1),
                        tile_position=(0, 0), skip_group_check=True,
                    )

        def squash(it, h):
            """v[:, half] = squash(scale * s_ps[:, half]), tiny b-part ops."""
            scale = 0.1 if it == 0 else 1.0
            o5 = slice(5 * h, 5 * h + 5)
            nc.scalar.activation(s2[:, o5, :], s_ps[:, o5, :], ACTF.Square,
                                 scale=scale)
            nc.vector.tensor_reduce(sq[:, o5], s2[:, o5, :], axis=AX.X,
                                    op=ALU.add)
            nc.scalar.activation(t1[:, o5], sq[:, o5], ACTF.Ln, bias=eps1)
            nc.scalar.activation(den[:, o5], t1[:, o5], ACTF.Exp, scale=0.5)
            nc.vector.tensor_scalar_add(t1[:, o5], sq[:, o5], 1.0)
            nc.vector.tensor_tensor(den[:, o5], den[:, o5], t1[:, o5],
                                    op=ALU.mult)
            nc.vector.reciprocal(rcp[:, o5], den[:, o5])
            nc.vector.tensor_tensor(ff[:, o5], sq[:, o5], rcp[:, o5],
                                    op=ALU.mult)
            if it == 0:
                nc.vector.tensor_scalar_mul(ff[:, o5], ff[:, o5], scale)
            nc.vector.tensor_tensor(
                v_f[:, o5, :], s_ps[:, o5, :],
                ff[:, o5].unsqueeze(2).broadcast_to([64, 5, J]),
                op=ALU.mult)
            nc.vector.tensor_copy(
                v_b[:, o5, :].rearrange("p o j -> p (o j)"),
                v_f[:, o5, :].rearrange("p o j -> p (o j)"))

        # half-h (g, sl) slots are disjoint: h0 -> sl0 strips + (g0, sl1);
        # h1 -> (g1..3, sl1) + (g0..1, sl2).
        def transposes(h):
            vt = psVT.tile([128, 3, B], BF16, name="vt", tag="vt0",
                           bufs=1)
            for o in range(5 * h, 5 * h + 5):
                g, sl = o % 4, o // 4
                nc.tensor.matmul(
                    vt[32 * g : 32 * g + 16, sl, :],
                    v_b[:, o, :], id64,
                    is_transpose=True, tile_position=(0, 32 * g),
                )
            for o in range(5 * h, 5 * h + 5):
                g, sl = o % 4, o // 4
                if VJB_ENG == "a":
                    nc.scalar.copy(v_jb[32 * g : 32 * g + 16, sl, :],
                                   vt[32 * g : 32 * g + 16, sl, :])
                else:
                    nc.vector.tensor_copy(
                        v_jb[32 * g : 32 * g + 16, sl, :],
                        vt[32 * g : 32 * g + 16, sl, :])

        flat = lambda ap: ap.rearrange("p t b -> p (t b)")
        flat3 = lambda ap: ap.rearrange("p a b -> p (a b)")

        def emit_G(o, route):
            """G^T chunks for o; returns the ug tile being filled."""
            g, sl = o % 4, o // 4
            ug = ugp.tile([128, KC, B], BF16, name="ug", tag="ug")
            gsb = None
            if route != "b":
                gsb = gsbp.tile([128, KC, B], BF16, name="gsb", tag="gsb")

            def gmm(pg, kk, kc):
                nc.tensor.matmul(
                    pg[:, kk, :],
                    wb2[32 * g : 32 * g + 16, sl, kc, :],
                    v_jb[32 * g : 32 * g + 16, sl, :],
                    start=True, stop=True,
                    tile_position=(32 * g, 0),
                )

            if route == "6":
                # 16-kc psG tiles: (Act, Act, DVE, DVE, Act-half); Pool
                # multiplies the Act-copied parts.
                for ti in range(5):
                    k0 = 16 * ti
                    nk = 16 if ti < 4 else 8
                    pg = psG.tile([128, 16, B], F32, name="pg", tag="pg")
                    for kk in range(nk):
                        gmm(pg, kk, k0 + kk)
                    slk = slice(k0, k0 + nk)
                    if ti in (0, 1, 4):
                        nc.scalar.copy(flat3(gsb[:, slk, :]),
                                       flat3(pg[:, 0:nk, :]))
                    else:
                        nc.vector.tensor_tensor(
                            flat3(ug[:, slk, :]), flat3(pg[:, 0:nk, :]),
                            flat3(ui2[:, slk, :]), op=ALU.mult)
                nc.gpsimd.tensor_tensor(
                    flat3(ug[:, 0:32, :]), flat3(gsb[:, 0:32, :]),
                    flat3(ui2[:, 0:32, :]), op=ALU.mult)
                nc.gpsimd.tensor_tensor(
                    flat3(ug[:, 64:72, :]), flat3(gsb[:, 64:72, :]),
                    flat3(ui2[:, 64:72, :]), op=ALU.mult)
                return ug

            nA = int(M_H[o]) if route in ("m", "M") else 9
            horder = list(range(9))
            if DVE_FIRST and route in ("m", "M"):
                horder = list(range(nA, 9)) + list(range(nA))
            for h in horder:
                pg = psG.tile([128, 8, B], F32, name="pg", tag="pg")
                for kk in range(8):
                    gmm(pg, kk, 8 * h + kk)
                sl8 = slice(8 * h, 8 * h + 8)
                if route in ("a", "A") or (route in ("m", "M") and h < nA):
                    nc.scalar.copy(flat3(gsb[:, sl8, :]),
                                   flat3(pg[:, 0:8, :]))
                else:  # DVE straight from PSUM
                    nc.vector.tensor_tensor(
                        flat3(ug[:, sl8, :]), flat3(pg[:, 0:8, :]),
                        flat3(ui2[:, sl8, :]), op=ALU.mult)
            if route in ("a", "A"):
                meng = nc.gpsimd if route == "A" else nc.vector
                meng.tensor_tensor(flat3(ug), flat3(gsb), flat3(ui2),
                                   op=ALU.mult)
            elif route in ("m", "M"):
                meng = nc.gpsimd if route == "M" else nc.vector
                if SPLIT_MULT and nA >= 3:
                    # two halves so the first can run while the later Act
                    # copies are still in flight
                    cut = 8 * (nA // 2 + 1)
                    for sl_ in (slice(0, cut), slice(cut, 8 * nA)):
                        meng.tensor_tensor(
                            flat3(ug[:, sl_, :]), flat3(gsb[:, sl_, :]),
                            flat3(ui2[:, sl_, :]), op=ALU.mult)
                else:
                    hA = slice(0, 8 * nA)
                    meng.tensor_tensor(
                        flat3(ug[:, hA, :]), flat3(gsb[:, hA, :]),
                        flat3(ui2[:, hA, :]), op=ALU.mult)
            return ug

        def emit_ds(o, ug):
            """delta[o] = sum_d ug chunks.  Returns (psum_tile|None, ug)."""
            eng = DS_ENG[o]
            if eng == "T":  # PE identity-matmul chains into PSUM f32
                ds = psDS.tile([128, T, B], F32, name="ds", tag="ds")
                for d in range(D):
                    nc.tensor.matmul(
                        flat3(ds[:, 0:8, :]), id128,
                        flat3(ug[:, d * T : d * T + 8, :]),
                        start=(d == 0), stop=(d == D - 1),
                        tile_position=(0, 0), skip_group_check=True,
                    )
                for d in range(D):
                    nc.tensor.matmul(
                        ds[:, 8, :], id128, ug[:, d * T + 8, :],
                        start=(d == 0), stop=(d == D - 1),
                        tile_position=(0, 0), skip_group_check=True,
                    )
                return ds, ug
            if eng == "x":
                # each fold split across DVE and Pool (disjoint halves)
                for w in (36, 18, 9):
                    cut = (w * 5) // 8
                    nc.vector.tensor_tensor(
                        flat3(ug[:, 0:cut, :]), flat3(ug[:, 0:cut, :]),
                        flat3(ug[:, w : w + cut, :]), op=ALU.add)
                    nc.gpsimd.tensor_tensor(
                        flat3(ug[:, cut:w, :]), flat3(ug[:, cut:w, :]),
                        flat3(ug[:, w + cut : 2 * w, :]), op=ALU.add)
                return None, ug
            ve = nc.vector if eng == "v" else nc.gpsimd
            # in-place bf16 fold tree: 72 -> 36 -> 18 -> 9 chunks
            for w in (36, 18, 9):
                ve.tensor_tensor(flat3(ug[:, 0:w, :]), flat3(ug[:, 0:w, :]),
                                 flat3(ug[:, w : 2 * w, :]), op=ALU.add)
            return None, ug

        def emit_exp(o, dsug, r):
            ds, ug = dsug
            src = flat3(ds) if ds is not None else flat3(ug[:, 0:T, :])
            if r == 0:
                nc.scalar.activation(flat(e[:, o]), src, ACTF.Exp)
            else:
                et = etp.tile([128, T, B], BF16, name="et", tag="et")
                nc.scalar.activation(flat(et), src, ACTF.Exp)
                ech = E_ENG[o]
                if ech == "s":
                    ecut = (B * 5) // 8
                    nc.vector.tensor_tensor(
                        e[:, o, :, 0:ecut], e[:, o, :, 0:ecut],
                        et[:, :, 0:ecut], op=ALU.mult)
                    nc.gpsimd.tensor_tensor(
                        e[:, o, :, ecut:B], e[:, o, :, ecut:B],
                        et[:, :, ecut:B], op=ALU.mult)
                else:
                    eeng = nc.gpsimd if ech == "g" else nc.vector
                    eeng.tensor_tensor(flat(e[:, o]), flat(e[:, o]),
                                       flat(et), op=ALU.mult)

        def emit_zpair(q):
            """partial softmax sums on Pool, overlapped with m2."""
            zq = zp.tile([128, T, B], BF16, name="zq", tag=f"z{q}", bufs=1)
            nc.gpsimd.tensor_tensor(flat(zq), flat(e[:, 2 * q]),
                                    flat(e[:, 2 * q + 1]), op=ALU.add)
            _zpart.append(zq)
            if q in (1, 3):  # fold pairs into quads as soon as available
                zz = zp.tile([128, T, B], BF16, name="zz", tag=f"zz{q}",
                             bufs=1)
                nc.gpsimd.tensor_tensor(flat(zz), flat(_zpart[-2]),
                                        flat(_zpart[-1]), op=ALU.add)
                _zquad.append(zz)

        DS_SHIFT = int(os.environ.get("DS_SHIFT", "3"))

        def m2(r, it):
            """delta_o for all o -> e (pass r), software-pipelined.
            Caller has emitted squash(it,0)+transposes(0); squash/transposes
            of the second half are interleaved after G(1)."""
            ugs = {}
            dss = {}
            for step in range(O + DS_SHIFT + 1):
                if step < O:
                    ugs[step] = emit_G(step, ROUTES[step])
                if step == 1:
                    squash(it, 1)
                    transposes(1)
                if 0 <= step - DS_SHIFT < O:
                    dss[step - DS_SHIFT] = emit_ds(
                        step - DS_SHIFT, ugs.pop(step - DS_SHIFT))
                if 0 <= step - DS_SHIFT - 1 < O:
                    oo = step - DS_SHIFT - 1
                    emit_exp(oo, dss.pop(oo), r)
                    if oo % 2 == 1:
                        emit_zpair(oo // 2)
                if step == 3 and len(_zpart) >= 2:
                    pass

        def softmax_tail():
            """finish Z = sum_o e; rz = 1/Z (bf16)."""
            za = zp.tile([128, T, B], BF16, name="za", tag="za")
            nc.vector.tensor_tensor(flat(za), flat(_zquad[0]),
                                    flat(_zquad[1]), op=ALU.add)
            nc.vector.tensor_tensor(flat(za), flat(za), flat(_zpart[4]),
                                    op=ALU.add)
            with nc.allow_low_precision("softmax normalizer, 2e-2 tolerance"):
                nc.vector.reciprocal(flat(rzb), flat(za))
            _zpart.clear()
            _zquad.clear()

        def m1_B(it):
            """s[b, o, j] = sum_{i,d} (c_o u) W for all o.
            squash/transposes of half 0 are emitted after o=4's chain."""
            for o in range(O):
                co = cop.tile([128, T, B], BF16, name="co", tag="co")
                cch = C_ENG[(it - 1) * O + o]
                if cch == "s":
                    ccut = (B * 5) // 8
                    nc.vector.tensor_tensor(
                        co[:, :, 0:ccut], e[:, o, :, 0:ccut],
                        rzb[:, :, 0:ccut], op=ALU.mult)
                    nc.gpsimd.tensor_tensor(
                        co[:, :, ccut:B], e[:, o, :, ccut:B],
                        rzb[:, :, ccut:B], op=ALU.mult)
                else:
                    ceng = nc.gpsimd if cch == "g" else nc.vector
                    ceng.tensor_tensor(flat(co), flat(e[:, o]), flat(rzb),
                                       op=ALU.mult)
                cu = cup.tile([128, T, D, B], BF16, name="cu", tag="cu")
                ch = CU_ENG[(it - 1) * O + o]
                if ch == "s":
                    # split the multiply across DVE and Pool (rate-matched)
                    cut = CU_CUT
                    nc.vector.tensor_tensor(
                        cu[:, :, :, 0:cut],
                        co[:, :, 0:cut].unsqueeze(2).broadcast_to(
                            [128, T, D, cut]),
                        ui[:, :, :, 0:cut], op=ALU.mult)
                    nc.gpsimd.tensor_tensor(
                        cu[:, :, :, cut:B],
                        co[:, :, cut:B].unsqueeze(2).broadcast_to(
                            [128, T, D, B - cut]),
                        ui[:, :, :, cut:B], op=ALU.mult)
                else:
                    cueng = nc.gpsimd if ch == "g" else nc.vector
                    cueng.tensor_tensor(
                        cu[:, :, :, :],
                        co.unsqueeze(2).broadcast_to([128, T, D, B]),
                        ui[:, :, :, :], op=ALU.mult)
                for t in range(T):
                    for d in range(D):
                        td = t * D + d
                        nc.tensor.matmul(
                            s_ps[:, o, :], cu[:, t, d, :], ws[:, t, d, o, :],
                            start=(td == 0), stop=(td == KC - 1),
                            tile_position=(0, 0), skip_group_check=True,
                        )
                if o == 4:
                    squash(it, 0)
                    if it < 2:
                        transposes(0)
                    else:
                        nc.sync.dma_start(out=vout_d[:, 0:5, :],
                                          in_=v_f[:, 0:5, :])

        _zpart = []
        _zquad = []

        # ========================= flow =========================
        W1 = int(os.environ.get("WARM1", "25"))
        W2 = int(os.environ.get("WARM2", "10"))
        m1_A_chain(0)
        squash(0, 0)
        m1_A_chain(1)
        transposes(0)
        for r in range(2):
            m2(r, r)
            pe_keepalive(W1)
            softmax_tail()
            m1_B(r + 1)
            pe_keepalive(W2)
        squash(2, 1)
        nc.sync.dma_start(out=vout_d[:, 5:10, :], in_=v_f[:, 5:10, :])

    nc.finalize()
    return nc


def _host_prep(u, weights):
    """Per-core input maps. u [512,1152,8] f32, weights [1152,10,16,8] f32."""
    W = np.asarray(weights, dtype=np.float32)
    u = np.asarray(u, dtype=np.float32)
    # ws[p, t, d, o, j] = W[t*128+p, o, j, d]
    ws = np.ascontiguousarray(
        W.reshape(T, 128, O, J, D).transpose(1, 0, 4, 2, 3)
    ).astype(bfnp)
    # wb2[32g+jj, sl, kc, m] = W[c*128+m, o, jj, d], kc = d*T + c
    wt = W.reshape(T, 128, O, J, D)  # [c, m, o, j, d]
    wb2 = np.zeros((128, 3, KC, 128), dtype=bfnp)
    for o in range(O):
        g, sl = o % 4, o // 4
        blk = wt[:, :, o, :, :].transpose(2, 3, 0, 1)  # [j, d, c, m]
        wb2[32 * g : 32 * g + 16, sl] = blk.reshape(J, KC, 128).astype(bfnp)
    id128 = np.eye(128, dtype=np.float32).astype(bfnp)
    id64 = np.eye(64, dtype=np.float32).astype(bfnp)

    base = {"ws": ws, "wb2": wb2, "id128": id128, "id64": id64}
    in_maps = []
    for c in range(N_CORES):
        uc = u[c * B : (c + 1) * B]  # [64, 1152, 8]
        ur = uc.reshape(B, T, 128, D)
        ui = np.ascontiguousarray(ur.transpose(2, 1, 3, 0)).astype(bfnp)
        # ui2[p, kc, b] = u[b, c*128+p, d], kc = d*T + c
        ui2 = np.ascontiguousarray(
            ur.transpose(2, 3, 1, 0).reshape(128, D * T, B)
        ).astype(bfnp)
        in_maps.append({**base, "ui": ui, "ui2": ui2})
    return in_maps


def kernel(u, weights):
    if "nc" not in _cache:
        _cache["nc"] = build_nc()
    nc = _cache["nc"]
    in_maps = _host_prep(u, weights)
    res = run_bass_kernel_spmd(nc, in_maps, core_ids=list(range(N_CORES)))
    out = np.concatenate([res.results[c]["vout"] for c in range(N_CORES)], axis=0)
    return out.astype(np.float32)


if __name__ == "__main__":
    rng = np.random.default_rng(0)
    u = rng.standard_normal((512, 1152, 8), dtype=np.float32)
    w = (rng.standard_normal((1152, 10, 16, 8)) * 0.1).astype(np.float32)
    v = kernel(u, w)
    print("out", v.shape, v.dtype, np.abs(v).max())
